# revision 38
# baseline (speedup 1.0000x reference)
"""AttentionPairBias kernel for Trainium2, 8-core SPMD.

Math (per batch=1):
  pn        = LayerNorm(pairwise) * gamma + beta                  [N, N, 128]
  attn_bias = einsum('ijp,ph->hij', pn, W_bias)                   [16, N, N]
  q,k,v     = single @ Wq/Wk/Wv  (split into 16 heads of 64)
  scores    = q k^T / sqrt(64) + attn_bias ; attn = softmax_j
  o         = attn @ v ; out = (o * sigmoid(single@Wg + bg)) @ Wo [N, 1024]

Sharding: rows of i (queries) across 8 cores; k/v compute replicated.

Engine plan: pairwise is pre-cast to bf16 on the host and streamed over
BOTH the SP (HWDGE) and Pool (SWDGE) DMA queues; PE transposes it to
[p,(j,i)]. PSUM->SBUF copies and squares split ACT/DVE (GPSIMD cannot
touch PSUM and its compute is slow on real HW, so Pool stays DMA-only);
LN stats reduce on PE via ones-matmul columns of the projection. The
q/k/v/g projections are emitted interleaved with the pairwise stream
(a_steps) so their matmuls fill PE while DMA paces phase B. Attention
runs with TRANSPOSED scores [j,i]: the softmax sum and the pair-bias add
both become PE matmuls (bias^T via identity-rhs matmul), so attn@v needs
no transpose and ACT only does one exp per head; 1/sum is folded into
the gate via a PE row-broadcast. The last j-quarter's LN correction is
deferred into the head loop to keep it off the B->C critical path.
LayerNorm is folded as a post-projection affine:
  bias[i,j,h] = rinv*(x@W_eff) - (rinv*mu)*colsum(W_eff)
(+beta@W_bias is constant over j so it cancels in softmax).
"""

import numpy as np
import ml_dtypes

import concourse.bacc as bacc
import concourse.bass as bass
import concourse.tile as tile
import concourse.mybir as mybir
from concourse.bass_utils import run_bass_kernel_spmd
from concourse.masks import make_identity

N, DIM, HEADS, DHEAD, DPAIR = 1024, 1024, 16, 64, 128
NCORES = 8
IBLK = N // NCORES  # 128
EPS = 1e-5

# tuning knobs
DMA_J = 32        # j-columns per pairwise DMA
QSP = frozenset(range(7, 32, 2))  # late odd chunks on SP (weights go first), rest Pool
QACT = frozenset()                # pairwise chunks on ACT queue
CP_ACT = 9        # pair copies: pjc%16 < CP_ACT -> ACT else DVE
SQ_DVE = 12       # pair squares: pjc%16 < SQ_DVE -> DVE else ACT

F32 = mybir.dt.float32
BF16 = mybir.dt.bfloat16
AX = mybir.AxisListType
AF = mybir.ActivationFunctionType
BFNP = ml_dtypes.bfloat16


def _bcast_free(ap, count, where=-1):
    """Append a zero-stride broadcast dim of length `count` to an AP."""
    return bass.AP(tensor=ap.tensor, offset=ap.offset, ap=list(ap.ap) + [[0, count]])


def _insert_bcast(ap, count, pos):
    """Insert a zero-stride broadcast dim of length `count` at free-dim
    position `pos` (0 = right after the partition dim)."""
    l = list(ap.ap)
    l.insert(1 + pos, [0, count])
    return bass.AP(tensor=ap.tensor, offset=ap.offset, ap=l)


def _swap_free(ap):
    """Swap the two free dims of a 3D AP (iteration-order change)."""
    l = list(ap.ap)
    assert len(l) == 3
    return bass.AP(tensor=ap.tensor, offset=ap.offset, ap=[l[0], l[2], l[1]])


def build_program(reps=1, **knobs):
    global DMA_J, QSP, QACT, CP_ACT, CP_POOL, SQ_DVE
    for k, v in knobs.items():
        if v is not None:
            globals()[k.upper()] = v
    nc = bacc.Bacc("TRN2", target_bir_lowering=False, debug=False)

    pair = nc.dram_tensor("pair", [IBLK, N, DPAIR], BF16, kind="ExternalInput")
    sT = nc.dram_tensor("sT", [DIM, N], BF16, kind="ExternalInput")
    sTi = nc.dram_tensor("sTi", [DIM, IBLK], BF16, kind="ExternalInput")
    wq = nc.dram_tensor("wq", [DIM, DIM], BF16, kind="ExternalInput")
    wk = nc.dram_tensor("wk", [DIM, DIM], BF16, kind="ExternalInput")
    wv = nc.dram_tensor("wv", [DIM, DIM], BF16, kind="ExternalInput")
    wg = nc.dram_tensor("wg", [DIM, DIM], BF16, kind="ExternalInput")
    wo = nc.dram_tensor("wo", [DIM, DIM], BF16, kind="ExternalInput")
    weff = nc.dram_tensor("weff", [DPAIR, HEADS + 1], BF16, kind="ExternalInput")
    colw = nc.dram_tensor("colw", [128, HEADS], F32, kind="ExternalInput")
    bgt = nc.dram_tensor("bgt", [128, 8], F32, kind="ExternalInput")
    out = nc.dram_tensor("out", [IBLK, DIM], F32, kind="ExternalOutput")

    CT = DIM // 128  # 8 contraction tiles

    with tile.TileContext(nc) as tc:
        with tc.tile_pool(name="consts", bufs=1) as consts, \
             tc.tile_pool(name="persist", bufs=1) as pers:
            ident = consts.tile([128, 128], BF16, tag="ident", name="ident")
            make_identity(nc, ident)
            ones1 = consts.tile([128, 1], BF16, tag="ones1", name="ones1")
            nc.vector.memset(ones1, 1.0)
            onesrow = consts.tile([1, 128], F32, tag="onesrow", name="onesrow")
            nc.vector.memset(onesrow, 1.0)
            weff_sb = consts.tile([DPAIR, HEADS + 1], BF16, tag="weff", name="weff")
            nc.sync.dma_start(out=weff_sb, in_=weff[:, :])
            colw_sb = consts.tile([128, HEADS], F32, tag="colw", name="colw")
            nc.sync.dma_start(out=colw_sb, in_=colw[:, :])
            bgt_sb = consts.tile([128, 8], F32, tag="bgt", name="bgt")
            nc.sync.dma_start(out=bgt_sb, in_=bgt[:, :])
            eps4 = consts.tile([128, 1], F32, tag="eps4", name="eps4")
            nc.vector.memset(eps4, EPS)
            zero1 = consts.tile([128, 1], F32, tag="zero1", name="zero1")
            nc.vector.memset(zero1, 0.0)

            for _rep in range(reps):
                # persistent tensors
                kT = [pers.tile([128, N], BF16, tag=f"kT{t}", name=f"kT{t}") for t in range(8)]
                vsb = [pers.tile([128, DIM], BF16, tag=f"v{t}", name=f"v{t}") for t in range(8)]
                qT = [pers.tile([128, IBLK], BF16, tag=f"qT{t}", name=f"qT{t}") for t in range(8)]
                gT = [pers.tile([128, IBLK], F32, tag=f"gT{t}", name=f"gT{t}") for t in range(8)]
                bias_h = pers.tile([128, HEADS, N], BF16, tag="biasH", name="biasH")
                # wo loads ride the otherwise-idle ACT HWDGE queue during B;
                # persistent tiles so they aren't gated on pb's SBUF freeing.
                wo_sb = [pers.tile([128, DIM], BF16, tag=f"wo{t}", name=f"wo{t}") for t in range(8)]
                for t in range(8):
                    nc.scalar.dma_start(out=wo_sb[t], in_=wo[t * 128:(t + 1) * 128, :])

                # ---------------- Phase B: pairwise LN + bias projection -----
                # (phase A's projections are emitted interleaved, see a_steps)
                with tc.tile_pool(name="pb", bufs=1) as pb, \
                     tc.tile_pool(name="psB", bufs=2, space="PSUM") as psB, \
                     tc.tile_pool(name="pa", bufs=1) as pa, \
                     tc.tile_pool(name="psA", bufs=2, space="PSUM") as psA:
                    # sums/sumsq interleaved: stats[:, j, 0]=sum, [:, j, 1]=sumsq
                    stats = pb.tile([128, N, 2], F32, tag="stats", name="stats")
                    rA = pers.tile([128, N], BF16, tag="rA", name="rA")
                    rm = pers.tile([128, N], BF16, tag="rm", name="rm")

                    def emit_quarter(qi):
                        """LN stats post-pass + affine correction of bias_h for
                        j-quarter qi, as soon as its sums/sumsq are complete."""
                        sl = slice(qi * 256, (qi + 1) * 256)
                        mu = pb.tile([128, 256], F32, tag="mu", name="mu", bufs=1)
                        v4 = pb.tile([128, 256], F32, tag="v4", name="v4", bufs=1)
                        d = pb.tile([128, 256], F32, tag="d", name="d", bufs=1)
                        nc.vector.tensor_scalar_mul(out=mu, in0=stats[:, sl, 0],
                                                    scalar1=1.0 / DPAIR)
                        nc.vector.tensor_scalar_mul(out=v4, in0=stats[:, sl, 1],
                                                    scalar1=1.0 / DPAIR)
                        nc.vector.tensor_mul(out=d, in0=mu, in1=mu)
                        nc.vector.tensor_sub(out=v4, in0=v4, in1=d)  # var
                        nc.scalar.activation(out=v4, in_=v4, func=AF.Sqrt,
                                             bias=eps4[:, 0:1], scale=1.0)
                        with nc.allow_low_precision(
                                reason="rinv in bf16: 0.4% rel on LN scale, "
                                       "well inside the 2e-2 gate"):
                            nc.vector.reciprocal(out=rA[:, sl], in_=v4)
                        nc.vector.tensor_mul(out=rm[:, sl], in0=mu, in1=rA[:, sl])
                        # the bias_h affine correction itself is deferred into
                        # phase C's head loop (DVE has slack there)

                    PAIR_J = 16           # j columns per processing unit
                    PAIR_PER_Q = 256 // PAIR_J
                    LAG = 1  # pairs of slack between transpose and pproj/psq

                    pending = []

                    def flush_pair():
                        """Emit pproj/psq + extracts for the oldest pending
                        16-j unit. Lagging these PE ops keeps the in-order PE
                        queue from stalling on the copy / square."""
                        pjc, oct, octsq = pending.pop(0)
                        j0 = pjc * PAIR_J
                        # pproj[:, j, 0:17] = [x@Weff | x@ones]; [:, j, 17] = x^2@ones
                        pproj = psB.tile([128, PAIR_J, HEADS + 2], F32, tag="pproj", bufs=2, name="pproj")
                        for jj in range(PAIR_J):
                            nc.tensor.matmul(pproj[:, jj, 0:HEADS + 1],
                                             oct[:, jj, :], weff_sb,
                                             start=True, stop=True)
                            nc.tensor.matmul(pproj[:, jj, HEADS + 1:HEADS + 2],
                                             octsq[:, jj, :], ones1,
                                             start=True, stop=True)
                        bsl = bias_h[:, :, j0:j0 + PAIR_J]
                        if pjc % 2:
                            nc.scalar.copy(out=_swap_free(bsl),
                                           in_=pproj[:, :, 0:HEADS])
                        else:
                            nc.vector.tensor_copy(out=_swap_free(bsl),
                                                  in_=pproj[:, :, 0:HEADS])
                        nc.vector.tensor_copy(out=stats[:, j0:j0 + PAIR_J, :],
                                              in_=pproj[:, :, HEADS:HEADS + 2])
                        if (pjc + 1) % PAIR_PER_Q == 0:
                            emit_quarter(pjc // PAIR_PER_Q)

                    NCH = N // DMA_J
                    # pairwise chunk -> DMA queue. SP is a free sequencer; its
                    # queue also carries the phase-A weight loads (emitted
                    # early via a_steps). Pool/ACT DMAs block their engine for
                    # the whole transfer, so balance against compute load.
                    x_tiles = {}

                    def issue_dma(dc):
                        x = pb.tile([128, DMA_J, DPAIR], BF16, tag="x", bufs=5, name="x")
                        eng = nc.sync if dc in QSP else (
                            nc.scalar if dc in QACT else nc.gpsimd)
                        eng.dma_start(
                            out=x, in_=pair[:, dc * DMA_J:(dc + 1) * DMA_J, :])
                        x_tiles[dc] = x

                    # ---- phase A, emitted in steps interleaved with B ----
                    def a_steps():
                        s_sb, si_sb = [], []
                        for ct in range(CT):
                            s = pa.tile([128, N], BF16, tag=f"s{ct}", name=f"s{ct}")
                            nc.sync.dma_start(out=s, in_=sT[ct * 128:(ct + 1) * 128, :])
                            s_sb.append(s)
                            si = pa.tile([128, IBLK], BF16, tag=f"si{ct}", name=f"si{ct}")
                            nc.sync.dma_start(out=si, in_=sTi[ct * 128:(ct + 1) * 128, :])
                            si_sb.append(si)
                        yield
                        # kT[t] = (Wk^T @ single^T)[rows t*128...]
                        wsb = [pa.tile([128, DIM], BF16, tag=f"w{ct}", name=f"w{ct}") for ct in range(CT)]
                        for ct in range(CT):
                            nc.sync.dma_start(out=wsb[ct], in_=wk[ct * 128:(ct + 1) * 128, :])
                        yield
                        for t in range(8):
                            for jh in range(2):
                                ps = psA.tile([128, 512], F32, tag="mmA", name="mmA")
                                for ct in range(CT):
                                    nc.tensor.matmul(
                                        ps, wsb[ct][:, t * 128:(t + 1) * 128],
                                        s_sb[ct][:, jh * 512:(jh + 1) * 512],
                                        start=(ct == 0), stop=(ct == CT - 1))
                                dst = kT[t][:, jh * 512:(jh + 1) * 512]
                                nc.scalar.copy(out=dst, in_=ps)
                            yield
                        # v[t] = (single @ Wv)[rows t*128...]   (natural layout)
                        wsb = [pa.tile([128, DIM], BF16, tag=f"w{ct}", name=f"w{ct}") for ct in range(CT)]
                        for ct in range(CT):
                            nc.sync.dma_start(out=wsb[ct], in_=wv[ct * 128:(ct + 1) * 128, :])
                        yield
                        for t in range(8):
                            for vh in range(2):
                                ps = psA.tile([128, 512], F32, tag="mmA", name="mmA")
                                for ct in range(CT):
                                    nc.tensor.matmul(
                                        ps, s_sb[ct][:, t * 128:(t + 1) * 128],
                                        wsb[ct][:, vh * 512:(vh + 1) * 512],
                                        start=(ct == 0), stop=(ct == CT - 1))
                                dst = vsb[t][:, vh * 512:(vh + 1) * 512]
                                nc.scalar.copy(out=dst, in_=ps)
                            yield
                        # qT[t] = (Wq^T @ single^T)[rows t*128, iblk] (Wq pre-scaled)
                        wsb = [pa.tile([128, DIM], BF16, tag=f"w{ct}", name=f"w{ct}") for ct in range(CT)]
                        for ct in range(CT):
                            nc.sync.dma_start(out=wsb[ct], in_=wq[ct * 128:(ct + 1) * 128, :])
                        yield
                        for t in range(8):
                            ps = psA.tile([128, IBLK], F32, tag="mmA", name="mmA")
                            for ct in range(CT):
                                nc.tensor.matmul(
                                    ps, wsb[ct][:, t * 128:(t + 1) * 128], si_sb[ct],
                                    start=(ct == 0), stop=(ct == CT - 1))
                            nc.scalar.copy(out=qT[t], in_=ps)
                            if t % 2:
                                yield
                        # gT[t] = sigmoid((Wg^T @ single^T)[rows t*128, iblk] + bg)
                        wsb = [pa.tile([128, DIM], BF16, tag=f"w{ct}", name=f"w{ct}") for ct in range(CT)]
                        for ct in range(CT):
                            nc.sync.dma_start(out=wsb[ct], in_=wg[ct * 128:(ct + 1) * 128, :])
                        yield
                        for t in range(8):
                            ps = psA.tile([128, IBLK], F32, tag="mmA", name="mmA")
                            for ct in range(CT):
                                nc.tensor.matmul(
                                    ps, wsb[ct][:, t * 128:(t + 1) * 128], si_sb[ct],
                                    start=(ct == 0), stop=(ct == CT - 1))
                            nc.scalar.activation(out=gT[t], in_=ps, func=AF.Sigmoid,
                                                 bias=bgt_sb[:, t:t + 1], scale=1.0)
                            if t % 2:
                                yield

                    agen = a_steps()
                    issue_dma(0)
                    issue_dma(1)
                    issue_dma(2)
                    issue_dma(3)
                    for dc in range(NCH):
                        x = x_tiles.pop(dc)
                        for sc in range(DMA_J // PAIR_J):
                            pjc = dc * (DMA_J // PAIR_J) + sc
                            xs = x[:, sc * PAIR_J:(sc + 1) * PAIR_J, :]
                            poct = psB.tile([128, PAIR_J, 128], BF16, tag="poct", bufs=2, name="poct")
                            for jj in range(PAIR_J):
                                nc.tensor.transpose(poct[:, jj, :], xs[:, jj, :], ident)
                            if sc == 0 and dc + 4 < NCH:
                                issue_dma(dc + 4)
                            oct = pb.tile([128, PAIR_J, 128], BF16, tag="oct", bufs=LAG + 2, name="oct")
                            m = (pjc * 5) % 16  # stride-5 spreads the split
                            if m < CP_ACT:
                                nc.scalar.copy(out=oct.bitcast(F32),
                                               in_=poct.bitcast(F32))
                            else:
                                nc.vector.tensor_copy(out=oct.bitcast(F32),
                                                      in_=poct.bitcast(F32))
                            octsq = pb.tile([128, PAIR_J, 128], BF16, tag="octsq",
                                            bufs=LAG + 1, name="octsq")
                            # GPSIMD can't touch PSUM and is slow; squares read
                            # the SBUF copy on DVE/ACT
                            if m < SQ_DVE:
                                nc.vector.tensor_mul(out=octsq, in0=oct, in1=oct)
                            else:
                                nc.scalar.activation(out=octsq, in_=oct,
                                                     func=AF.Square)
                            pending.append((pjc, oct, octsq))
                            if len(pending) > LAG:
                                flush_pair()
                            if pjc % 2 == 1:
                                next(agen, None)
                    while pending:
                        flush_pair()
                    for _ in agen:
                        pass

                # ---------------- Phase C: attention (transposed scores) -----
                with tc.tile_pool(name="pc", bufs=1) as pc, \
                     tc.tile_pool(name="psC", bufs=2, space="PSUM") as psC:
                    og = [pc.tile([128, IBLK], BF16, tag=f"og{t}", name=f"og{t}") for t in range(8)]

                    # Software-pipelined over heads: head h's ssum/av (PE ops
                    # that wait on exp_h) are emitted AFTER head h+1's kq/bias
                    # matmuls, so the in-order PE queue never stalls on ACT.
                    state = {}  # t -> (rsb, ot_ps)

                    def finish_head(h, expT):
                        t = h // 2
                        off = 64 * (h % 2)
                        if h % 2 == 0:
                            rsb = psC.tile([128, IBLK], F32, tag="ot", bufs=2, name="rsb")
                            ot_ps = psC.tile([128, IBLK], F32, tag="ot", bufs=2, name="ot")
                            state[t] = (rsb, ot_ps)
                        rsb, ot_ps = state[t]
                        ssb = psC.tile([1, 128], F32, tag="ssb", bufs=2, name="ssb")
                        for jb in range(8):
                            nc.tensor.matmul(ssb, ones1, expT[:, jb, :],
                                             start=(jb == 0), stop=(jb == 7))
                        rs = pc.tile([1, 128], F32, tag="rs", bufs=3, name="rs")
                        nc.vector.reciprocal(out=rs, in_=ssb)
                        nc.tensor.matmul(rsb[off:off + 64, :],
                                         onesrow[:, 0:64], rs,
                                         start=True, stop=True)
                        for jt in range(8):
                            nc.tensor.matmul(
                                ot_ps[off:off + 64, :],
                                vsb[jt][:, h * 64:(h + 1) * 64], expT[:, jt, :],
                                start=(jt == 0), stop=(jt == 7))
                        if h % 2 == 1:
                            nc.vector.tensor_mul(out=gT[t], in0=gT[t], in1=rsb)
                            nc.vector.tensor_mul(out=og[t], in0=ot_ps, in1=gT[t])

                    prev = None
                    SL3 = slice(0, 1024)
                    for h in range(HEADS):
                        t = h // 2
                        off = 64 * (h % 2)
                        # deferred LN correction of quarter 3 for this head
                        nc.vector.tensor_mul(out=bias_h[:, h, SL3],
                                             in0=bias_h[:, h, SL3],
                                             in1=rA[:, SL3])
                        t2h = pc.tile([128, 1024], BF16, tag="t2h", name="t2h",
                                      bufs=2)
                        nc.vector.tensor_scalar_mul(
                            out=t2h, in0=rm[:, SL3],
                            scalar1=colw_sb[:, h:h + 1])
                        nc.vector.tensor_add(out=bias_h[:, h, SL3],
                                             in0=bias_h[:, h, SL3], in1=t2h)
                        # scT[j, i] for j-block jb: k^T q + bias^T (identity-rhs)
                        scT = psC.tile([128, 8, 128], F32, tag="scT", bufs=2, name="scT")
                        for jb in range(8):
                            nc.tensor.matmul(
                                scT[:, jb, :],
                                kT[t][off:off + 64, jb * 128:(jb + 1) * 128],
                                qT[t][off:off + 64, :], start=True, stop=False)
                            nc.tensor.matmul(
                                scT[:, jb, :],
                                bias_h[:, h, jb * 128:(jb + 1) * 128], ident,
                                start=False, stop=True)
                        # scores are O(10): exp without max-subtraction is safe in
                        # f32/bf16 range, softmax is shift-invariant.
                        expT = pc.tile([128, 8, 128], BF16, tag="expT", bufs=3, name="expT")
                        nc.scalar.activation(out=expT, in_=scT, func=AF.Exp,
                                             bias=zero1[:, 0:1], scale=1.0)
                        if prev is not None:
                            finish_head(*prev)
                        prev = (h, expT)
                    finish_head(*prev)

                    # out = og^T @ Wo
                    out_sb = pc.tile([128, DIM], F32, tag="out_sb", name="out_sb")
                    for eh in range(2):
                        ps = psC.tile([128, 512], F32, tag="scT", bufs=2, name="po")
                        for t in range(8):
                            nc.tensor.matmul(
                                ps, og[t], wo_sb[t][:, eh * 512:(eh + 1) * 512],
                                start=(t == 0), stop=(t == 7))
                        nc.scalar.copy(out=out_sb[:, eh * 512:(eh + 1) * 512], in_=ps)
                    nc.sync.dma_start(out=out[:, :], in_=out_sb)

    nc.compile()
    return nc


_CACHE = {}


def _prep_inputs(single_repr, pairwise_repr, ln_gamma, ln_beta, W_bias,
                 Wq, Wk, Wv, Wg, bg, Wo):
    sr = np.asarray(single_repr, np.float32).reshape(N, DIM)
    pw = np.asarray(pairwise_repr, np.float32).reshape(N, N, DPAIR).astype(BFNP)
    gamma = np.asarray(ln_gamma, np.float32)
    Wb = np.asarray(W_bias, np.float32)
    weff = gamma[:, None] * Wb                                   # [128, 16]
    colw = np.ascontiguousarray(
        np.broadcast_to(-weff.sum(0)[None, :], (128, HEADS))).astype(np.float32)
    weff17 = np.concatenate(
        [weff, np.ones((DPAIR, 1), np.float32)], axis=1)         # [128, 17]
    sT_np = np.ascontiguousarray(sr.T).astype(BFNP)              # [DIM, N]
    scale = DHEAD ** -0.5
    common = {
        "sT": sT_np,
        "wq": (np.asarray(Wq, np.float32) * scale).astype(BFNP),
        "wk": np.asarray(Wk, np.float32).astype(BFNP),
        "wv": np.asarray(Wv, np.float32).astype(BFNP),
        "wg": np.asarray(Wg, np.float32).astype(BFNP),
        "wo": np.asarray(Wo, np.float32).astype(BFNP),
        "weff": weff17.astype(BFNP),
        "colw": colw,
        "bgt": np.ascontiguousarray(
            np.asarray(bg, np.float32).reshape(8, 128).T),
    }
    in_maps = []
    for c in range(NCORES):
        m = dict(common)
        m["pair"] = pw[c * IBLK:(c + 1) * IBLK]
        m["sTi"] = np.ascontiguousarray(sT_np[:, c * IBLK:(c + 1) * IBLK])
        in_maps.append(m)
    return in_maps


def kernel(single_repr, pairwise_repr, ln_gamma, ln_beta, W_bias,
           Wq, Wk, Wv, Wg, bg, Wo, _trace=False):
    if "nc" not in _CACHE:
        _CACHE["nc"] = build_program()
    nc = _CACHE["nc"]
    in_maps = _prep_inputs(single_repr, pairwise_repr, ln_gamma, ln_beta,
                           W_bias, Wq, Wk, Wv, Wg, bg, Wo)
    res = run_bass_kernel_spmd(nc, in_maps, core_ids=list(range(NCORES)),
                               trace=_trace)
    out = np.concatenate([res.results[c]["out"] for c in range(NCORES)], axis=0)
    if _trace:
        kernel.last_result = res
    return out.reshape(1, N, DIM).astype(np.float32)


# revision 41
# speedup vs baseline: 1.0792x; 1.0792x over previous
"""AttentionPairBias kernel for Trainium2, 8-core SPMD.

Math (per batch=1):
  pn        = LayerNorm(pairwise) * gamma + beta                  [N, N, 128]
  attn_bias = einsum('ijp,ph->hij', pn, W_bias)                   [16, N, N]
  q,k,v     = single @ Wq/Wk/Wv  (split into 16 heads of 64)
  scores    = q k^T / sqrt(64) + attn_bias ; attn = softmax_j
  o         = attn @ v ; out = (o * sigmoid(single@Wg + bg)) @ Wo [N, 1024]

Sharding: rows of i (queries) across 8 cores; k/v compute replicated.

Engine plan: pairwise is pre-cast to bf16 on the host and streamed over
BOTH the SP (HWDGE) and Pool (SWDGE) DMA queues; PE transposes it to
[p,(j,i)]. PSUM->SBUF copies and squares split ACT/DVE (GPSIMD cannot
touch PSUM and its compute is slow on real HW, so Pool stays DMA-only);
LN stats reduce on PE via ones-matmul columns of the projection. The
q/k/v/g projections are emitted interleaved with the pairwise stream
(a_steps) so their matmuls fill PE while DMA paces phase B. Attention
runs with TRANSPOSED scores [j,i]: the softmax sum and the pair-bias add
both become PE matmuls (bias^T via identity-rhs matmul), so attn@v needs
no transpose and ACT only does one exp per head; 1/sum is folded into
the gate via a PE row-broadcast. The last j-quarter's LN correction is
deferred into the head loop to keep it off the B->C critical path.
LayerNorm is folded as a post-projection affine:
  bias[i,j,h] = rinv*(x@W_eff) - (rinv*mu)*colsum(W_eff)
(+beta@W_bias is constant over j so it cancels in softmax).
"""

import numpy as np
import ml_dtypes

import concourse.bacc as bacc
import concourse.bass as bass
import concourse.tile as tile
import concourse.mybir as mybir
from concourse.bass_utils import run_bass_kernel_spmd
from concourse.masks import make_identity

N, DIM, HEADS, DHEAD, DPAIR = 1024, 1024, 16, 64, 128
NCORES = 8
IBLK = N // NCORES  # 128
EPS = 1e-5

# tuning knobs
DMA_J = 32        # j-columns per pairwise DMA
QSP = frozenset(range(7, 32, 2))  # late odd chunks on SP (weights go first), rest Pool
QACT = frozenset()                # pairwise chunks on ACT queue
CP_ACT = 9        # pair copies: pjc%16 < CP_ACT -> ACT else DVE
SQ_DVE = 12       # pair squares: pjc%16 < SQ_DVE -> DVE else ACT

F32 = mybir.dt.float32
BF16 = mybir.dt.bfloat16
AX = mybir.AxisListType
AF = mybir.ActivationFunctionType
BFNP = ml_dtypes.bfloat16


def _bcast_free(ap, count, where=-1):
    """Append a zero-stride broadcast dim of length `count` to an AP."""
    return bass.AP(tensor=ap.tensor, offset=ap.offset, ap=list(ap.ap) + [[0, count]])


def _insert_bcast(ap, count, pos):
    """Insert a zero-stride broadcast dim of length `count` at free-dim
    position `pos` (0 = right after the partition dim)."""
    l = list(ap.ap)
    l.insert(1 + pos, [0, count])
    return bass.AP(tensor=ap.tensor, offset=ap.offset, ap=l)


def _swap_free(ap):
    """Swap the two free dims of a 3D AP (iteration-order change)."""
    l = list(ap.ap)
    assert len(l) == 3
    return bass.AP(tensor=ap.tensor, offset=ap.offset, ap=[l[0], l[2], l[1]])


def build_program(reps=1, **knobs):
    global DMA_J, QSP, QACT, CP_ACT, CP_POOL, SQ_DVE
    for k, v in knobs.items():
        if v is not None:
            globals()[k.upper()] = v
    nc = bacc.Bacc("TRN2", target_bir_lowering=False, debug=False)

    pair = nc.dram_tensor("pair", [IBLK, N, DPAIR], BF16, kind="ExternalInput")
    sT = nc.dram_tensor("sT", [DIM, N], BF16, kind="ExternalInput")
    sTi = nc.dram_tensor("sTi", [DIM, IBLK], BF16, kind="ExternalInput")
    wq = nc.dram_tensor("wq", [DIM, DIM], BF16, kind="ExternalInput")
    wk = nc.dram_tensor("wk", [DIM, DIM], BF16, kind="ExternalInput")
    wv = nc.dram_tensor("wv", [DIM, DIM], BF16, kind="ExternalInput")
    wg = nc.dram_tensor("wg", [DIM, DIM], BF16, kind="ExternalInput")
    wo = nc.dram_tensor("wo", [DIM, DIM], BF16, kind="ExternalInput")
    weff = nc.dram_tensor("weff", [DPAIR, HEADS + 1], BF16, kind="ExternalInput")
    colw = nc.dram_tensor("colw", [128, HEADS], F32, kind="ExternalInput")
    bgt = nc.dram_tensor("bgt", [128, 8], F32, kind="ExternalInput")
    out = nc.dram_tensor("out", [IBLK, DIM], F32, kind="ExternalOutput")

    CT = DIM // 128  # 8 contraction tiles

    with tile.TileContext(nc) as tc:
        with tc.tile_pool(name="consts", bufs=1) as consts, \
             tc.tile_pool(name="persist", bufs=1) as pers:
            ident = consts.tile([128, 128], BF16, tag="ident", name="ident")
            make_identity(nc, ident)
            ones1 = consts.tile([128, 1], BF16, tag="ones1", name="ones1")
            nc.vector.memset(ones1, 1.0)
            onesrow = consts.tile([1, 128], F32, tag="onesrow", name="onesrow")
            nc.vector.memset(onesrow, 1.0)
            weff_sb = consts.tile([DPAIR, HEADS + 1], BF16, tag="weff", name="weff")
            nc.sync.dma_start(out=weff_sb, in_=weff[:, :])
            colw_sb = consts.tile([128, HEADS], F32, tag="colw", name="colw")
            nc.sync.dma_start(out=colw_sb, in_=colw[:, :])
            bgt_sb = consts.tile([128, 8], F32, tag="bgt", name="bgt")
            nc.sync.dma_start(out=bgt_sb, in_=bgt[:, :])
            eps4 = consts.tile([128, 1], F32, tag="eps4", name="eps4")
            nc.vector.memset(eps4, EPS)
            zero1 = consts.tile([128, 1], F32, tag="zero1", name="zero1")
            nc.vector.memset(zero1, 0.0)

            for _rep in range(reps):
                # persistent tensors
                kT = [pers.tile([128, N], BF16, tag=f"kT{t}", name=f"kT{t}") for t in range(8)]
                vsb = [pers.tile([128, DIM], BF16, tag=f"v{t}", name=f"v{t}") for t in range(8)]
                qT = [pers.tile([128, IBLK], BF16, tag=f"qT{t}", name=f"qT{t}") for t in range(8)]
                gT = [pers.tile([128, IBLK], F32, tag=f"gT{t}", name=f"gT{t}") for t in range(8)]
                bias_h = pers.tile([128, HEADS, N], BF16, tag="biasH", name="biasH")
                # wo loads ride the otherwise-idle ACT HWDGE queue during B;
                # persistent tiles so they aren't gated on pb's SBUF freeing.
                wo_sb = [pers.tile([128, DIM], BF16, tag=f"wo{t}", name=f"wo{t}") for t in range(8)]
                for t in range(8):
                    nc.scalar.dma_start(out=wo_sb[t], in_=wo[t * 128:(t + 1) * 128, :])

                # ---------------- Phase B: pairwise LN + bias projection -----
                # (phase A's projections are emitted interleaved, see a_steps)
                with tc.tile_pool(name="pb", bufs=1) as pb, \
                     tc.tile_pool(name="psB", bufs=2, space="PSUM") as psB, \
                     tc.tile_pool(name="pa", bufs=1) as pa, \
                     tc.tile_pool(name="psA", bufs=2, space="PSUM") as psA:
                    # sums/sumsq interleaved: stats[:, j, 0]=sum, [:, j, 1]=sumsq
                    stats = pb.tile([128, N, 2], F32, tag="stats", name="stats")
                    rA = pers.tile([128, N], BF16, tag="rA", name="rA")
                    rm = pers.tile([128, N], BF16, tag="rm", name="rm")

                    def emit_quarter(qi):
                        """LN stats post-pass + affine correction of bias_h for
                        j-quarter qi, as soon as its sums/sumsq are complete."""
                        sl = slice(qi * 256, (qi + 1) * 256)
                        mu = pb.tile([128, 256], F32, tag="mu", name="mu", bufs=1)
                        v4 = pb.tile([128, 256], F32, tag="v4", name="v4", bufs=1)
                        d = pb.tile([128, 256], F32, tag="d", name="d", bufs=1)
                        nc.vector.tensor_scalar_mul(out=mu, in0=stats[:, sl, 0],
                                                    scalar1=1.0 / DPAIR)
                        nc.vector.tensor_scalar_mul(out=v4, in0=stats[:, sl, 1],
                                                    scalar1=1.0 / DPAIR)
                        nc.vector.tensor_mul(out=d, in0=mu, in1=mu)
                        nc.vector.tensor_sub(out=v4, in0=v4, in1=d)  # var
                        nc.scalar.activation(out=v4, in_=v4, func=AF.Sqrt,
                                             bias=eps4[:, 0:1], scale=1.0)
                        with nc.allow_low_precision(
                                reason="rinv in bf16: 0.4% rel on LN scale, "
                                       "well inside the 2e-2 gate"):
                            nc.vector.reciprocal(out=rA[:, sl], in_=v4)
                        nc.vector.tensor_mul(out=rm[:, sl], in0=mu, in1=rA[:, sl])
                        # the bias_h affine correction itself is deferred into
                        # phase C's head loop (DVE has slack there)

                    PAIR_J = 16           # j columns per processing unit
                    PAIR_PER_Q = 256 // PAIR_J
                    LAG = 1  # pairs of slack between transpose and pproj/psq

                    pending = []

                    def flush_pair():
                        """Emit pproj/psq + extracts for the oldest pending
                        16-j unit. Lagging these PE ops keeps the in-order PE
                        queue from stalling on the copy / square."""
                        pjc, oct, octsq = pending.pop(0)
                        j0 = pjc * PAIR_J
                        # pproj[:, j, 0:17] = [x@Weff | x@ones]; [:, j, 17] = x^2@ones
                        pproj = psB.tile([128, PAIR_J, HEADS + 2], F32, tag="pproj", bufs=2, name="pproj")
                        for jj in range(PAIR_J):
                            nc.tensor.matmul(pproj[:, jj, 0:HEADS + 1],
                                             oct[:, jj, :], weff_sb,
                                             start=True, stop=True)
                            nc.tensor.matmul(pproj[:, jj, HEADS + 1:HEADS + 2],
                                             octsq[:, jj, :], ones1,
                                             start=True, stop=True)
                        bsl = bias_h[:, :, j0:j0 + PAIR_J]
                        if pjc % 2:
                            nc.scalar.copy(out=_swap_free(bsl),
                                           in_=pproj[:, :, 0:HEADS])
                        else:
                            nc.vector.tensor_copy(out=_swap_free(bsl),
                                                  in_=pproj[:, :, 0:HEADS])
                        nc.vector.tensor_copy(out=stats[:, j0:j0 + PAIR_J, :],
                                              in_=pproj[:, :, HEADS:HEADS + 2])
                        if (pjc + 1) % PAIR_PER_Q == 0:
                            emit_quarter(pjc // PAIR_PER_Q)

                    NCH = N // DMA_J
                    # pairwise chunk -> DMA queue. SP is a free sequencer; its
                    # queue also carries the phase-A weight loads (emitted
                    # early via a_steps). Pool/ACT DMAs block their engine for
                    # the whole transfer, so balance against compute load.
                    x_tiles = {}

                    def issue_dma(dc):
                        x = pb.tile([128, DMA_J, DPAIR], BF16, tag="x", bufs=5, name="x")
                        eng = nc.sync if dc in QSP else (
                            nc.scalar if dc in QACT else nc.gpsimd)
                        eng.dma_start(
                            out=x, in_=pair[:, dc * DMA_J:(dc + 1) * DMA_J, :])
                        x_tiles[dc] = x

                    # ---- phase A, emitted in steps interleaved with B ----
                    def a_steps():
                        s_sb, si_sb = [], []
                        for ct in range(CT):
                            s = pa.tile([128, N], BF16, tag=f"s{ct}", name=f"s{ct}")
                            nc.sync.dma_start(out=s, in_=sT[ct * 128:(ct + 1) * 128, :])
                            s_sb.append(s)
                            si = pa.tile([128, IBLK], BF16, tag=f"si{ct}", name=f"si{ct}")
                            nc.sync.dma_start(out=si, in_=sTi[ct * 128:(ct + 1) * 128, :])
                            si_sb.append(si)
                        yield
                        # kT[t] = (Wk^T @ single^T)[rows t*128...]
                        wsb = [pa.tile([128, DIM], BF16, tag=f"w{ct}", name=f"w{ct}") for ct in range(CT)]
                        for ct in range(CT):
                            nc.sync.dma_start(out=wsb[ct], in_=wk[ct * 128:(ct + 1) * 128, :])
                        yield
                        for t in range(8):
                            for jh in range(2):
                                ps = psA.tile([128, 512], F32, tag="mmA", name="mmA")
                                for ct in range(CT):
                                    nc.tensor.matmul(
                                        ps, wsb[ct][:, t * 128:(t + 1) * 128],
                                        s_sb[ct][:, jh * 512:(jh + 1) * 512],
                                        start=(ct == 0), stop=(ct == CT - 1))
                                dst = kT[t][:, jh * 512:(jh + 1) * 512]
                                nc.scalar.copy(out=dst, in_=ps)
                            yield
                        # v[t] = (single @ Wv)[rows t*128...]   (natural layout)
                        wsb = [pa.tile([128, DIM], BF16, tag=f"w{ct}", name=f"w{ct}") for ct in range(CT)]
                        for ct in range(CT):
                            nc.sync.dma_start(out=wsb[ct], in_=wv[ct * 128:(ct + 1) * 128, :])
                        yield
                        for t in range(8):
                            for vh in range(2):
                                ps = psA.tile([128, 512], F32, tag="mmA", name="mmA")
                                for ct in range(CT):
                                    nc.tensor.matmul(
                                        ps, s_sb[ct][:, t * 128:(t + 1) * 128],
                                        wsb[ct][:, vh * 512:(vh + 1) * 512],
                                        start=(ct == 0), stop=(ct == CT - 1))
                                dst = vsb[t][:, vh * 512:(vh + 1) * 512]
                                nc.scalar.copy(out=dst, in_=ps)
                            yield
                        # qT[t] = (Wq^T @ single^T)[rows t*128, iblk] (Wq pre-scaled)
                        wsb = [pa.tile([128, DIM], BF16, tag=f"w{ct}", name=f"w{ct}") for ct in range(CT)]
                        for ct in range(CT):
                            nc.sync.dma_start(out=wsb[ct], in_=wq[ct * 128:(ct + 1) * 128, :])
                        yield
                        for t in range(8):
                            ps = psA.tile([128, IBLK], F32, tag="mmA", name="mmA")
                            for ct in range(CT):
                                nc.tensor.matmul(
                                    ps, wsb[ct][:, t * 128:(t + 1) * 128], si_sb[ct],
                                    start=(ct == 0), stop=(ct == CT - 1))
                            nc.scalar.copy(out=qT[t], in_=ps)
                            if t % 2:
                                yield
                        # gT[t] = sigmoid((Wg^T @ single^T)[rows t*128, iblk] + bg)
                        wsb = [pa.tile([128, DIM], BF16, tag=f"w{ct}", name=f"w{ct}") for ct in range(CT)]
                        for ct in range(CT):
                            nc.sync.dma_start(out=wsb[ct], in_=wg[ct * 128:(ct + 1) * 128, :])
                        yield
                        for t in range(8):
                            ps = psA.tile([128, IBLK], F32, tag="mmA", name="mmA")
                            for ct in range(CT):
                                nc.tensor.matmul(
                                    ps, wsb[ct][:, t * 128:(t + 1) * 128], si_sb[ct],
                                    start=(ct == 0), stop=(ct == CT - 1))
                            nc.scalar.activation(out=gT[t], in_=ps, func=AF.Sigmoid,
                                                 bias=bgt_sb[:, t:t + 1], scale=1.0)
                            if t % 2:
                                yield

                    agen = a_steps()
                    issue_dma(0)
                    issue_dma(1)
                    issue_dma(2)
                    issue_dma(3)
                    for dc in range(NCH):
                        x = x_tiles.pop(dc)
                        for sc in range(DMA_J // PAIR_J):
                            pjc = dc * (DMA_J // PAIR_J) + sc
                            xs = x[:, sc * PAIR_J:(sc + 1) * PAIR_J, :]
                            poct = psB.tile([128, PAIR_J, 128], BF16, tag="poct", bufs=2, name="poct")
                            for jj in range(PAIR_J):
                                nc.tensor.transpose(poct[:, jj, :], xs[:, jj, :], ident)
                            if sc == 0 and dc + 4 < NCH:
                                issue_dma(dc + 4)
                            oct = pb.tile([128, PAIR_J, 128], BF16, tag="oct", bufs=LAG + 2, name="oct")
                            m = (pjc * 5) % 16  # stride-5 spreads the split
                            if m < CP_ACT:
                                nc.scalar.copy(out=oct.bitcast(F32),
                                               in_=poct.bitcast(F32))
                            else:
                                nc.vector.tensor_copy(out=oct.bitcast(F32),
                                                      in_=poct.bitcast(F32))
                            octsq = pb.tile([128, PAIR_J, 128], BF16, tag="octsq",
                                            bufs=LAG + 1, name="octsq")
                            # all squares read the SBUF copy: dual-PSUM reads
                            # are illegal on DVE, and reading poct on ACT
                            # extends the PSUM tile's lifetime (transpose stalls)
                            if m < SQ_DVE:
                                nc.vector.tensor_mul(out=octsq, in0=oct, in1=oct)
                            else:
                                nc.scalar.activation(out=octsq, in_=oct,
                                                     func=AF.Square)
                            pending.append((pjc, oct, octsq))
                            if len(pending) > LAG:
                                flush_pair()
                            if pjc % 2 == 1:
                                next(agen, None)
                    while pending:
                        flush_pair()
                    for _ in agen:
                        pass

                # ---------------- Phase C: attention (transposed scores) -----
                with tc.tile_pool(name="pc", bufs=1) as pc, \
                     tc.tile_pool(name="psC", bufs=2, space="PSUM") as psC:
                    og = [pc.tile([128, IBLK], BF16, tag=f"og{t}", name=f"og{t}") for t in range(8)]

                    # Software-pipelined over heads: head h's ssum/av (PE ops
                    # that wait on exp_h) are emitted AFTER head h+1's kq/bias
                    # matmuls, so the in-order PE queue never stalls on ACT.
                    state = {}  # t -> (rsb, ot_ps)

                    def finish_head(h, expT):
                        t = h // 2
                        off = 64 * (h % 2)
                        if h % 2 == 0:
                            rsb = psC.tile([128, IBLK], F32, tag="ot", bufs=2, name="rsb")
                            ot_ps = psC.tile([128, IBLK], F32, tag="ot", bufs=2, name="ot")
                            state[t] = (rsb, ot_ps)
                        rsb, ot_ps = state[t]
                        ssb = psC.tile([1, 128], F32, tag="ssb", bufs=2, name="ssb")
                        for jb in range(8):
                            nc.tensor.matmul(ssb, ones1, expT[:, jb, :],
                                             start=(jb == 0), stop=(jb == 7))
                        rs = pc.tile([1, 128], F32, tag="rs", bufs=3, name="rs")
                        nc.vector.reciprocal(out=rs, in_=ssb)
                        nc.tensor.matmul(rsb[off:off + 64, :],
                                         onesrow[:, 0:64], rs,
                                         start=True, stop=True)
                        for jt in range(8):
                            nc.tensor.matmul(
                                ot_ps[off:off + 64, :],
                                vsb[jt][:, h * 64:(h + 1) * 64], expT[:, jt, :],
                                start=(jt == 0), stop=(jt == 7))
                        if h % 2 == 1:
                            nc.vector.tensor_mul(out=gT[t], in0=gT[t], in1=rsb)
                            nc.vector.tensor_mul(out=og[t], in0=ot_ps, in1=gT[t])

                    prev = None
                    SL3 = slice(0, 1024)
                    for h in range(HEADS):
                        t = h // 2
                        off = 64 * (h % 2)
                        # deferred LN correction of quarter 3 for this head
                        nc.vector.tensor_mul(out=bias_h[:, h, SL3],
                                             in0=bias_h[:, h, SL3],
                                             in1=rA[:, SL3])
                        t2h = pc.tile([128, 1024], BF16, tag="t2h", name="t2h",
                                      bufs=2)
                        nc.vector.tensor_scalar_mul(
                            out=t2h, in0=rm[:, SL3],
                            scalar1=colw_sb[:, h:h + 1])
                        nc.vector.tensor_add(out=bias_h[:, h, SL3],
                                             in0=bias_h[:, h, SL3], in1=t2h)
                        # scT[j, i] for j-block jb: k^T q + bias^T (identity-rhs)
                        scT = psC.tile([128, 8, 128], F32, tag="scT", bufs=2, name="scT")
                        for jb in range(8):
                            nc.tensor.matmul(
                                scT[:, jb, :],
                                kT[t][off:off + 64, jb * 128:(jb + 1) * 128],
                                qT[t][off:off + 64, :], start=True, stop=False)
                            nc.tensor.matmul(
                                scT[:, jb, :],
                                bias_h[:, h, jb * 128:(jb + 1) * 128], ident,
                                start=False, stop=True)
                        # scores are O(10): exp without max-subtraction is safe in
                        # f32/bf16 range, softmax is shift-invariant.
                        expT = pc.tile([128, 8, 128], BF16, tag="expT", bufs=3, name="expT")
                        nc.scalar.activation(out=expT, in_=scT, func=AF.Exp,
                                             bias=zero1[:, 0:1], scale=1.0)
                        if prev is not None:
                            finish_head(*prev)
                        prev = (h, expT)
                    finish_head(*prev)

                    # out = og^T @ Wo
                    out_sb = pc.tile([128, DIM], F32, tag="out_sb", name="out_sb")
                    for eh in range(2):
                        ps = psC.tile([128, 512], F32, tag="scT", bufs=2, name="po")
                        for t in range(8):
                            nc.tensor.matmul(
                                ps, og[t], wo_sb[t][:, eh * 512:(eh + 1) * 512],
                                start=(t == 0), stop=(t == 7))
                        nc.scalar.copy(out=out_sb[:, eh * 512:(eh + 1) * 512], in_=ps)
                    nc.sync.dma_start(out=out[:, :], in_=out_sb)

    nc.compile()
    return nc


_CACHE = {}


def _prep_inputs(single_repr, pairwise_repr, ln_gamma, ln_beta, W_bias,
                 Wq, Wk, Wv, Wg, bg, Wo):
    sr = np.asarray(single_repr, np.float32).reshape(N, DIM)
    pw = np.asarray(pairwise_repr, np.float32).reshape(N, N, DPAIR).astype(BFNP)
    gamma = np.asarray(ln_gamma, np.float32)
    Wb = np.asarray(W_bias, np.float32)
    weff = gamma[:, None] * Wb                                   # [128, 16]
    colw = np.ascontiguousarray(
        np.broadcast_to(-weff.sum(0)[None, :], (128, HEADS))).astype(np.float32)
    weff17 = np.concatenate(
        [weff, np.ones((DPAIR, 1), np.float32)], axis=1)         # [128, 17]
    sT_np = np.ascontiguousarray(sr.T).astype(BFNP)              # [DIM, N]
    scale = DHEAD ** -0.5
    common = {
        "sT": sT_np,
        "wq": (np.asarray(Wq, np.float32) * scale).astype(BFNP),
        "wk": np.asarray(Wk, np.float32).astype(BFNP),
        "wv": np.asarray(Wv, np.float32).astype(BFNP),
        "wg": np.asarray(Wg, np.float32).astype(BFNP),
        "wo": np.asarray(Wo, np.float32).astype(BFNP),
        "weff": weff17.astype(BFNP),
        "colw": colw,
        "bgt": np.ascontiguousarray(
            np.asarray(bg, np.float32).reshape(8, 128).T),
    }
    in_maps = []
    for c in range(NCORES):
        m = dict(common)
        m["pair"] = pw[c * IBLK:(c + 1) * IBLK]
        m["sTi"] = np.ascontiguousarray(sT_np[:, c * IBLK:(c + 1) * IBLK])
        in_maps.append(m)
    return in_maps


def kernel(single_repr, pairwise_repr, ln_gamma, ln_beta, W_bias,
           Wq, Wk, Wv, Wg, bg, Wo, _trace=False):
    if "nc" not in _CACHE:
        _CACHE["nc"] = build_program()
    nc = _CACHE["nc"]
    in_maps = _prep_inputs(single_repr, pairwise_repr, ln_gamma, ln_beta,
                           W_bias, Wq, Wk, Wv, Wg, bg, Wo)
    res = run_bass_kernel_spmd(nc, in_maps, core_ids=list(range(NCORES)),
                               trace=_trace)
    out = np.concatenate([res.results[c]["out"] for c in range(NCORES)], axis=0)
    if _trace:
        kernel.last_result = res
    return out.reshape(1, N, DIM).astype(np.float32)


# revision 44
# speedup vs baseline: 1.1150x; 1.0332x over previous
"""AttentionPairBias kernel for Trainium2, 8-core SPMD.

Math (per batch=1):
  pn        = LayerNorm(pairwise) * gamma + beta                  [N, N, 128]
  attn_bias = einsum('ijp,ph->hij', pn, W_bias)                   [16, N, N]
  q,k,v     = single @ Wq/Wk/Wv  (split into 16 heads of 64)
  scores    = q k^T / sqrt(64) + attn_bias ; attn = softmax_j
  o         = attn @ v ; out = (o * sigmoid(single@Wg + bg)) @ Wo [N, 1024]

Sharding: rows of i (queries) across 8 cores; k/v compute replicated.

Engine plan: pairwise is pre-cast to bf16 on the host and streamed over
BOTH the SP (HWDGE) and Pool (SWDGE) DMA queues; PE transposes it to
[p,(j,i)]. PSUM->SBUF copies and squares split ACT/DVE (GPSIMD cannot
touch PSUM and its compute is slow on real HW, so Pool stays DMA-only);
LN stats reduce on PE via ones-matmul columns of the projection. The
q/k/v/g projections are emitted interleaved with the pairwise stream
(a_steps) so their matmuls fill PE while DMA paces phase B. Attention
runs with TRANSPOSED scores [j,i]: the softmax sum and the pair-bias add
both become PE matmuls (bias^T via identity-rhs matmul), so attn@v needs
no transpose and ACT only does one exp per head; 1/sum is folded into
the gate via a PE row-broadcast. The last j-quarter's LN correction is
deferred into the head loop to keep it off the B->C critical path.
LayerNorm is folded as a post-projection affine:
  bias[i,j,h] = rinv*(x@W_eff) - (rinv*mu)*colsum(W_eff)
(+beta@W_bias is constant over j so it cancels in softmax).
"""

import numpy as np
import ml_dtypes

import concourse.bacc as bacc
import concourse.bass as bass
import concourse.tile as tile
import concourse.mybir as mybir
from concourse.bass_utils import run_bass_kernel_spmd
from concourse.masks import make_identity

N, DIM, HEADS, DHEAD, DPAIR = 1024, 1024, 16, 64, 128
NCORES = 8
IBLK = N // NCORES  # 128
EPS = 1e-5

# tuning knobs
DMA_J = 32        # j-columns per pairwise DMA
QSP = frozenset(range(7, 32, 2))  # late odd chunks on SP (weights go first), rest Pool
QACT = frozenset()                # pairwise chunks on ACT queue
CP_ACT = 9        # pair copies: pjc%16 < CP_ACT -> ACT else DVE
SQ_DVE = 12       # pair squares: pjc%16 < SQ_DVE -> DVE else ACT

F32 = mybir.dt.float32
BF16 = mybir.dt.bfloat16
AX = mybir.AxisListType
AF = mybir.ActivationFunctionType
BFNP = ml_dtypes.bfloat16


def _bcast_free(ap, count, where=-1):
    """Append a zero-stride broadcast dim of length `count` to an AP."""
    return bass.AP(tensor=ap.tensor, offset=ap.offset, ap=list(ap.ap) + [[0, count]])


def _insert_bcast(ap, count, pos):
    """Insert a zero-stride broadcast dim of length `count` at free-dim
    position `pos` (0 = right after the partition dim)."""
    l = list(ap.ap)
    l.insert(1 + pos, [0, count])
    return bass.AP(tensor=ap.tensor, offset=ap.offset, ap=l)


def _swap_free(ap):
    """Swap the two free dims of a 3D AP (iteration-order change)."""
    l = list(ap.ap)
    assert len(l) == 3
    return bass.AP(tensor=ap.tensor, offset=ap.offset, ap=[l[0], l[2], l[1]])


def build_program(reps=1, **knobs):
    global DMA_J, QSP, QACT, CP_ACT, CP_POOL, SQ_DVE
    for k, v in knobs.items():
        if v is not None:
            globals()[k.upper()] = v
    nc = bacc.Bacc("TRN2", target_bir_lowering=False, debug=False)

    pair = nc.dram_tensor("pair", [IBLK, N, DPAIR], BF16, kind="ExternalInput")
    sT = nc.dram_tensor("sT", [DIM, N], BF16, kind="ExternalInput")
    sTi = nc.dram_tensor("sTi", [DIM, IBLK], BF16, kind="ExternalInput")
    wq = nc.dram_tensor("wq", [DIM, DIM], BF16, kind="ExternalInput")
    wk = nc.dram_tensor("wk", [DIM, DIM], BF16, kind="ExternalInput")
    wv = nc.dram_tensor("wv", [DIM, DIM], BF16, kind="ExternalInput")
    wg = nc.dram_tensor("wg", [DIM, DIM], BF16, kind="ExternalInput")
    wo = nc.dram_tensor("wo", [DIM, DIM], BF16, kind="ExternalInput")
    weff = nc.dram_tensor("weff", [DPAIR, HEADS + 1], BF16, kind="ExternalInput")
    colw = nc.dram_tensor("colw", [128, HEADS], F32, kind="ExternalInput")
    bgt = nc.dram_tensor("bgt", [128, 8], F32, kind="ExternalInput")
    out = nc.dram_tensor("out", [IBLK, DIM], F32, kind="ExternalOutput")

    CT = DIM // 128  # 8 contraction tiles

    with tile.TileContext(nc) as tc:
        with tc.tile_pool(name="consts", bufs=1) as consts, \
             tc.tile_pool(name="persist", bufs=1) as pers:
            ident = consts.tile([128, 128], BF16, tag="ident", name="ident")
            make_identity(nc, ident)
            ones1 = consts.tile([128, 1], BF16, tag="ones1", name="ones1")
            nc.vector.memset(ones1, 1.0)
            onesrow = consts.tile([1, 128], F32, tag="onesrow", name="onesrow")
            nc.vector.memset(onesrow, 1.0)
            weff_sb = consts.tile([DPAIR, HEADS + 1], BF16, tag="weff", name="weff")
            nc.sync.dma_start(out=weff_sb, in_=weff[:, :])
            colw_sb = consts.tile([128, HEADS], F32, tag="colw", name="colw")
            nc.sync.dma_start(out=colw_sb, in_=colw[:, :])
            bgt_sb = consts.tile([128, 8], F32, tag="bgt", name="bgt")
            nc.sync.dma_start(out=bgt_sb, in_=bgt[:, :])
            eps4 = consts.tile([128, 1], F32, tag="eps4", name="eps4")
            nc.vector.memset(eps4, EPS)
            zero1 = consts.tile([128, 1], F32, tag="zero1", name="zero1")
            nc.vector.memset(zero1, 0.0)

            for _rep in range(reps):
                # persistent tensors
                kT = [pers.tile([128, N], BF16, tag=f"kT{t}", name=f"kT{t}") for t in range(8)]
                vsb = [pers.tile([128, DIM], BF16, tag=f"v{t}", name=f"v{t}") for t in range(8)]
                qT = [pers.tile([128, IBLK], BF16, tag=f"qT{t}", name=f"qT{t}") for t in range(8)]
                gT = [pers.tile([128, IBLK], F32, tag=f"gT{t}", name=f"gT{t}") for t in range(8)]
                bias_h = pers.tile([128, HEADS, N], BF16, tag="biasH", name="biasH")
                # wo loads ride the otherwise-idle ACT HWDGE queue during B;
                # persistent tiles so they aren't gated on pb's SBUF freeing.
                wo_sb = [pers.tile([128, DIM], BF16, tag=f"wo{t}", name=f"wo{t}") for t in range(8)]
                for t in range(8):
                    nc.scalar.dma_start(out=wo_sb[t], in_=wo[t * 128:(t + 1) * 128, :])

                # ---------------- Phase B: pairwise LN + bias projection -----
                # (phase A's projections are emitted interleaved, see a_steps)
                with tc.tile_pool(name="pb", bufs=1) as pb, \
                     tc.tile_pool(name="psB", bufs=2, space="PSUM") as psB, \
                     tc.tile_pool(name="pa", bufs=1) as pa, \
                     tc.tile_pool(name="psA", bufs=2, space="PSUM") as psA:
                    # sums/sumsq interleaved: stats[:, j, 0]=sum, [:, j, 1]=sumsq
                    stats = pb.tile([128, N, 2], F32, tag="stats", name="stats")
                    rA = pers.tile([128, N], BF16, tag="rA", name="rA")
                    rm = pers.tile([128, N], BF16, tag="rm", name="rm")

                    def emit_quarter(qi):
                        """LN stats post-pass for j-quarter qi, as soon as its
                        sums/sumsq are complete."""
                        sl = slice(qi * 256, (qi + 1) * 256)
                        mu = pb.tile([128, 256], F32, tag="mu", name="mu", bufs=1)
                        v4 = pb.tile([128, 256], F32, tag="v4", name="v4", bufs=1)
                        d = pb.tile([128, 256], F32, tag="d", name="d", bufs=1)
                        nc.vector.tensor_scalar_mul(out=mu, in0=stats[:, sl, 0],
                                                    scalar1=1.0 / DPAIR)
                        nc.vector.tensor_scalar_mul(out=v4, in0=stats[:, sl, 1],
                                                    scalar1=1.0 / DPAIR)
                        nc.vector.tensor_mul(out=d, in0=mu, in1=mu)
                        nc.vector.tensor_sub(out=v4, in0=v4, in1=d)  # var
                        nc.scalar.activation(out=v4, in_=v4, func=AF.Sqrt,
                                             bias=eps4[:, 0:1], scale=1.0)
                        with nc.allow_low_precision(
                                reason="rinv in bf16: 0.4% rel on LN scale, "
                                       "well inside the 2e-2 gate"):
                            nc.vector.reciprocal(out=rA[:, sl], in_=v4)
                        nc.vector.tensor_mul(out=rm[:, sl], in0=mu, in1=rA[:, sl])
                        # the bias_h affine correction itself is deferred into
                        # phase C's head loop (DVE has slack there)

                    PAIR_J = 16           # j columns per processing unit
                    PAIR_PER_Q = 256 // PAIR_J
                    LAG = 1  # pairs of slack between transpose and pproj/psq

                    pending = []

                    def flush_pair():
                        """Emit pproj/psq + extracts for the oldest pending
                        16-j unit. Lagging these PE ops keeps the in-order PE
                        queue from stalling on the copy / square."""
                        pjc, oct, octsq = pending.pop(0)
                        j0 = pjc * PAIR_J
                        # pproj[:, j, 0:17] = [x@Weff | x@ones]; [:, j, 17] = x^2@ones
                        pproj = psB.tile([128, PAIR_J, HEADS + 2], F32, tag="pproj", bufs=2, name="pproj")
                        for jj in range(PAIR_J):
                            nc.tensor.matmul(pproj[:, jj, 0:HEADS + 1],
                                             oct[:, jj, :], weff_sb,
                                             start=True, stop=True)
                            nc.tensor.matmul(pproj[:, jj, HEADS + 1:HEADS + 2],
                                             octsq[:, jj, :], ones1,
                                             start=True, stop=True)
                        bsl = bias_h[:, :, j0:j0 + PAIR_J]
                        if pjc % 2:
                            nc.scalar.copy(out=_swap_free(bsl),
                                           in_=pproj[:, :, 0:HEADS])
                        else:
                            nc.vector.tensor_copy(out=_swap_free(bsl),
                                                  in_=pproj[:, :, 0:HEADS])
                        nc.vector.tensor_copy(out=stats[:, j0:j0 + PAIR_J, :],
                                              in_=pproj[:, :, HEADS:HEADS + 2])
                        if (pjc + 1) % PAIR_PER_Q == 0:
                            emit_quarter(pjc // PAIR_PER_Q)

                    NCH = N // DMA_J
                    # pairwise chunk -> DMA queue. SP is a free sequencer; its
                    # queue also carries the phase-A weight loads (emitted
                    # early via a_steps). Pool/ACT DMAs block their engine for
                    # the whole transfer, so balance against compute load.
                    x_tiles = {}

                    def issue_dma(dc):
                        x = pb.tile([128, DMA_J, DPAIR], BF16, tag="x", bufs=5, name="x")
                        eng = nc.sync if dc in QSP else (
                            nc.scalar if dc in QACT else nc.gpsimd)
                        eng.dma_start(
                            out=x, in_=pair[:, dc * DMA_J:(dc + 1) * DMA_J, :])
                        x_tiles[dc] = x

                    # ---- phase A, emitted in steps interleaved with B ----
                    def a_steps():
                        s_sb, si_sb = [], []
                        for ct in range(CT):
                            s = pa.tile([128, N], BF16, tag=f"s{ct}", name=f"s{ct}")
                            nc.sync.dma_start(out=s, in_=sT[ct * 128:(ct + 1) * 128, :])
                            s_sb.append(s)
                            si = pa.tile([128, IBLK], BF16, tag=f"si{ct}", name=f"si{ct}")
                            nc.sync.dma_start(out=si, in_=sTi[ct * 128:(ct + 1) * 128, :])
                            si_sb.append(si)
                        yield
                        # kT[t] = (Wk^T @ single^T)[rows t*128...]
                        wsb = [pa.tile([128, DIM], BF16, tag=f"w{ct}", name=f"w{ct}") for ct in range(CT)]
                        for ct in range(CT):
                            nc.sync.dma_start(out=wsb[ct], in_=wk[ct * 128:(ct + 1) * 128, :])
                        yield
                        for t in range(8):
                            for jh in range(2):
                                ps = psA.tile([128, 512], F32, tag="mmA", name="mmA")
                                for ct in range(CT):
                                    nc.tensor.matmul(
                                        ps, wsb[ct][:, t * 128:(t + 1) * 128],
                                        s_sb[ct][:, jh * 512:(jh + 1) * 512],
                                        start=(ct == 0), stop=(ct == CT - 1))
                                dst = kT[t][:, jh * 512:(jh + 1) * 512]
                                nc.scalar.copy(out=dst, in_=ps)
                            yield
                        # v[t] = (single @ Wv)[rows t*128...]   (natural layout)
                        wsb = [pa.tile([128, DIM], BF16, tag=f"w{ct}", name=f"w{ct}") for ct in range(CT)]
                        for ct in range(CT):
                            nc.sync.dma_start(out=wsb[ct], in_=wv[ct * 128:(ct + 1) * 128, :])
                        yield
                        for t in range(8):
                            for vh in range(2):
                                ps = psA.tile([128, 512], F32, tag="mmA", name="mmA")
                                for ct in range(CT):
                                    nc.tensor.matmul(
                                        ps, s_sb[ct][:, t * 128:(t + 1) * 128],
                                        wsb[ct][:, vh * 512:(vh + 1) * 512],
                                        start=(ct == 0), stop=(ct == CT - 1))
                                dst = vsb[t][:, vh * 512:(vh + 1) * 512]
                                nc.scalar.copy(out=dst, in_=ps)
                            yield
                        # qT[t] = (Wq^T @ single^T)[rows t*128, iblk] (Wq pre-scaled)
                        wsb = [pa.tile([128, DIM], BF16, tag=f"w{ct}", name=f"w{ct}") for ct in range(CT)]
                        for ct in range(CT):
                            nc.sync.dma_start(out=wsb[ct], in_=wq[ct * 128:(ct + 1) * 128, :])
                        yield
                        for t in range(8):
                            ps = psA.tile([128, IBLK], F32, tag="mmA", name="mmA")
                            for ct in range(CT):
                                nc.tensor.matmul(
                                    ps, wsb[ct][:, t * 128:(t + 1) * 128], si_sb[ct],
                                    start=(ct == 0), stop=(ct == CT - 1))
                            nc.scalar.copy(out=qT[t], in_=ps)
                            if t % 2:
                                yield
                        # gT[t] = sigmoid((Wg^T @ single^T)[rows t*128, iblk] + bg)
                        wsb = [pa.tile([128, DIM], BF16, tag=f"w{ct}", name=f"w{ct}") for ct in range(CT)]
                        for ct in range(CT):
                            nc.sync.dma_start(out=wsb[ct], in_=wg[ct * 128:(ct + 1) * 128, :])
                        yield
                        for t in range(8):
                            ps = psA.tile([128, IBLK], F32, tag="mmA", name="mmA")
                            for ct in range(CT):
                                nc.tensor.matmul(
                                    ps, wsb[ct][:, t * 128:(t + 1) * 128], si_sb[ct],
                                    start=(ct == 0), stop=(ct == CT - 1))
                            nc.scalar.activation(out=gT[t], in_=ps, func=AF.Sigmoid,
                                                 bias=bgt_sb[:, t:t + 1], scale=1.0)
                            if t % 2:
                                yield

                    agen = a_steps()
                    issue_dma(0)
                    issue_dma(1)
                    issue_dma(2)
                    issue_dma(3)
                    for dc in range(NCH):
                        x = x_tiles.pop(dc)
                        for sc in range(DMA_J // PAIR_J):
                            pjc = dc * (DMA_J // PAIR_J) + sc
                            xs = x[:, sc * PAIR_J:(sc + 1) * PAIR_J, :]
                            poct = psB.tile([128, PAIR_J, 128], BF16, tag="poct", bufs=2, name="poct")
                            for jj in range(PAIR_J):
                                nc.tensor.transpose(poct[:, jj, :], xs[:, jj, :], ident)
                            if sc == 0 and dc + 4 < NCH:
                                issue_dma(dc + 4)
                            oct = pb.tile([128, PAIR_J, 128], BF16, tag="oct", bufs=LAG + 2, name="oct")
                            m = (pjc * 5) % 16  # stride-5 spreads the split
                            if m < CP_ACT:
                                nc.scalar.copy(out=oct.bitcast(F32),
                                               in_=poct.bitcast(F32))
                            else:
                                nc.vector.tensor_copy(out=oct.bitcast(F32),
                                                      in_=poct.bitcast(F32))
                            octsq = pb.tile([128, PAIR_J, 128], BF16, tag="octsq",
                                            bufs=LAG + 1, name="octsq")
                            # all squares read the SBUF copy: dual-PSUM reads
                            # are illegal on DVE, and reading poct on ACT
                            # extends the PSUM tile's lifetime (transpose stalls)
                            if m < SQ_DVE:
                                nc.vector.tensor_mul(out=octsq, in0=oct, in1=oct)
                            else:
                                nc.scalar.activation(out=octsq, in_=oct,
                                                     func=AF.Square)
                            pending.append((pjc, oct, octsq))
                            if len(pending) > LAG:
                                flush_pair()
                            if pjc % 2 == 1:
                                next(agen, None)
                    while pending:
                        flush_pair()
                    for _ in agen:
                        pass

                # ---------------- Phase C: attention (transposed scores) -----
                with tc.tile_pool(name="pc", bufs=1) as pc, \
                     tc.tile_pool(name="psC", bufs=2, space="PSUM") as psC:
                    og = [pc.tile([128, IBLK], BF16, tag=f"og{t}", name=f"og{t}") for t in range(8)]

                    # Software-pipelined over heads: head h's ssum/av (PE ops
                    # that wait on exp_h) are emitted AFTER head h+1's kq/bias
                    # matmuls, so the in-order PE queue never stalls on ACT.
                    state = {}  # t -> (rsb, ot_ps)

                    def finish_head(h, expT):
                        t = h // 2
                        off = 64 * (h % 2)
                        if h % 2 == 0:
                            rsb = psC.tile([128, IBLK], F32, tag="ot", bufs=2, name="rsb")
                            ot_ps = psC.tile([128, IBLK], F32, tag="ot", bufs=2, name="ot")
                            state[t] = (rsb, ot_ps)
                        rsb, ot_ps = state[t]
                        ssb = psC.tile([1, 128], F32, tag="ssb", bufs=2, name="ssb")
                        for jb in range(8):
                            nc.tensor.matmul(ssb, ones1, expT[:, jb, :],
                                             start=(jb == 0), stop=(jb == 7))
                        rs = pc.tile([1, 128], F32, tag="rs", bufs=3, name="rs")
                        nc.vector.reciprocal(out=rs, in_=ssb)
                        nc.tensor.matmul(rsb[off:off + 64, :],
                                         onesrow[:, 0:64], rs,
                                         start=True, stop=True)
                        for jt in range(8):
                            nc.tensor.matmul(
                                ot_ps[off:off + 64, :],
                                vsb[jt][:, h * 64:(h + 1) * 64], expT[:, jt, :],
                                start=(jt == 0), stop=(jt == 7))
                        if h % 2 == 1:
                            nc.vector.tensor_mul(out=gT[t], in0=gT[t], in1=rsb)
                            nc.vector.tensor_mul(out=og[t], in0=ot_ps, in1=gT[t])

                    prev = None
                    SL3 = slice(0, 1024)
                    for h in range(HEADS):
                        t = h // 2
                        off = 64 * (h % 2)
                        # deferred LN correction of quarter 3 for this head
                        nc.vector.tensor_mul(out=bias_h[:, h, SL3],
                                             in0=bias_h[:, h, SL3],
                                             in1=rA[:, SL3])
                        t2h = pc.tile([128, 1024], BF16, tag="t2h", name="t2h",
                                      bufs=2)
                        nc.vector.tensor_scalar_mul(
                            out=t2h, in0=rm[:, SL3],
                            scalar1=colw_sb[:, h:h + 1])
                        nc.vector.tensor_add(out=bias_h[:, h, SL3],
                                             in0=bias_h[:, h, SL3], in1=t2h)
                        # scT[j, i] for j-block jb: k^T q + bias^T (identity-rhs)
                        scT = psC.tile([128, 8, 128], F32, tag="scT", bufs=2, name="scT")
                        for jb in range(8):
                            nc.tensor.matmul(
                                scT[:, jb, :],
                                kT[t][off:off + 64, jb * 128:(jb + 1) * 128],
                                qT[t][off:off + 64, :], start=True, stop=False)
                            nc.tensor.matmul(
                                scT[:, jb, :],
                                bias_h[:, h, jb * 128:(jb + 1) * 128], ident,
                                start=False, stop=True)
                        # scores are O(10): exp without max-subtraction is safe in
                        # f32/bf16 range, softmax is shift-invariant.
                        expT = pc.tile([128, 8, 128], BF16, tag="expT", bufs=3, name="expT")
                        nc.scalar.activation(out=expT, in_=scT, func=AF.Exp,
                                             bias=zero1[:, 0:1], scale=1.0)
                        if prev is not None:
                            finish_head(*prev)
                        prev = (h, expT)
                    finish_head(*prev)

                    # out = og^T @ Wo
                    out_sb = pc.tile([128, DIM], F32, tag="out_sb", name="out_sb")
                    for eh in range(2):
                        ps = psC.tile([128, 512], F32, tag="scT", bufs=2, name="po")
                        for t in range(8):
                            nc.tensor.matmul(
                                ps, og[t], wo_sb[t][:, eh * 512:(eh + 1) * 512],
                                start=(t == 0), stop=(t == 7))
                        nc.scalar.copy(out=out_sb[:, eh * 512:(eh + 1) * 512], in_=ps)
                    nc.sync.dma_start(out=out[:, :], in_=out_sb)

    nc.compile()
    return nc


_CACHE = {}


def _prep_inputs(single_repr, pairwise_repr, ln_gamma, ln_beta, W_bias,
                 Wq, Wk, Wv, Wg, bg, Wo):
    sr = np.asarray(single_repr, np.float32).reshape(N, DIM)
    pw = np.asarray(pairwise_repr, np.float32).reshape(N, N, DPAIR).astype(BFNP)
    gamma = np.asarray(ln_gamma, np.float32)
    Wb = np.asarray(W_bias, np.float32)
    weff = gamma[:, None] * Wb                                   # [128, 16]
    colw = np.ascontiguousarray(
        np.broadcast_to(-weff.sum(0)[None, :], (128, HEADS))).astype(np.float32)
    weff17 = np.concatenate(
        [weff, np.ones((DPAIR, 1), np.float32)], axis=1)         # [128, 17]
    sT_np = np.ascontiguousarray(sr.T).astype(BFNP)              # [DIM, N]
    scale = DHEAD ** -0.5
    common = {
        "sT": sT_np,
        "wq": (np.asarray(Wq, np.float32) * scale).astype(BFNP),
        "wk": np.asarray(Wk, np.float32).astype(BFNP),
        "wv": np.asarray(Wv, np.float32).astype(BFNP),
        "wg": np.asarray(Wg, np.float32).astype(BFNP),
        "wo": np.asarray(Wo, np.float32).astype(BFNP),
        "weff": weff17.astype(BFNP),
        "colw": colw,
        "bgt": np.ascontiguousarray(
            np.asarray(bg, np.float32).reshape(8, 128).T),
    }
    in_maps = []
    for c in range(NCORES):
        m = dict(common)
        m["pair"] = pw[c * IBLK:(c + 1) * IBLK]
        m["sTi"] = np.ascontiguousarray(sT_np[:, c * IBLK:(c + 1) * IBLK])
        in_maps.append(m)
    return in_maps


def kernel(single_repr, pairwise_repr, ln_gamma, ln_beta, W_bias,
           Wq, Wk, Wv, Wg, bg, Wo, _trace=False):
    if "nc" not in _CACHE:
        _CACHE["nc"] = build_program()
    nc = _CACHE["nc"]
    in_maps = _prep_inputs(single_repr, pairwise_repr, ln_gamma, ln_beta,
                           W_bias, Wq, Wk, Wv, Wg, bg, Wo)
    res = run_bass_kernel_spmd(nc, in_maps, core_ids=list(range(NCORES)),
                               trace=_trace)
    out = np.concatenate([res.results[c]["out"] for c in range(NCORES)], axis=0)
    if _trace:
        kernel.last_result = res
    return out.reshape(1, N, DIM).astype(np.float32)


# revision 48
# speedup vs baseline: 1.1669x; 1.0465x over previous
"""AttentionPairBias kernel for Trainium2, 8-core SPMD.

Math (per batch=1):
  pn        = LayerNorm(pairwise) * gamma + beta                  [N, N, 128]
  attn_bias = einsum('ijp,ph->hij', pn, W_bias)                   [16, N, N]
  q,k,v     = single @ Wq/Wk/Wv  (split into 16 heads of 64)
  scores    = q k^T / sqrt(64) + attn_bias ; attn = softmax_j
  o         = attn @ v ; out = (o * sigmoid(single@Wg + bg)) @ Wo [N, 1024]

Sharding: rows of i (queries) across 8 cores; k/v compute replicated.

Engine plan: pairwise is pre-cast to bf16 on the host and streamed over
BOTH the SP (HWDGE) and Pool (SWDGE) DMA queues; PE transposes it to
[p,(j,i)]. PSUM->SBUF copies and squares split ACT/DVE (GPSIMD cannot
touch PSUM and its compute is slow on real HW, so Pool stays DMA-only);
LN stats reduce on PE via ones-matmul columns of the projection. The
q/k/v/g projections are emitted interleaved with the pairwise stream
(a_steps) so their matmuls fill PE while DMA paces phase B. Attention
runs with TRANSPOSED scores [j,i]: the softmax sum and the pair-bias add
both become PE matmuls (bias^T via identity-rhs matmul), so attn@v needs
no transpose and ACT only does one exp per head; 1/sum is folded into
the gate via a PE row-broadcast. The last j-quarter's LN correction is
deferred into the head loop to keep it off the B->C critical path.
LayerNorm is folded as a post-projection affine:
  bias[i,j,h] = rinv*(x@W_eff) - (rinv*mu)*colsum(W_eff)
(+beta@W_bias is constant over j so it cancels in softmax).
"""

import numpy as np
import ml_dtypes

import concourse.bacc as bacc
import concourse.bass as bass
import concourse.tile as tile
import concourse.mybir as mybir
from concourse.bass_utils import run_bass_kernel_spmd
from concourse.masks import make_identity

N, DIM, HEADS, DHEAD, DPAIR = 1024, 1024, 16, 64, 128
NCORES = 8
IBLK = N // NCORES  # 128
EPS = 1e-5

# tuning knobs
DMA_J = 32        # j-columns per pairwise DMA
QSP = frozenset(range(7, 32, 2))  # late odd chunks on SP (weights go first), rest Pool
QACT = frozenset()                # pairwise chunks on ACT queue
CP_ACT = 9        # pair copies: pjc%16 < CP_ACT -> ACT else DVE
SQ_DVE = 12       # pair squares: pjc%16 < SQ_DVE -> DVE else ACT

F32 = mybir.dt.float32
BF16 = mybir.dt.bfloat16
AX = mybir.AxisListType
AF = mybir.ActivationFunctionType
BFNP = ml_dtypes.bfloat16


def _bcast_free(ap, count, where=-1):
    """Append a zero-stride broadcast dim of length `count` to an AP."""
    return bass.AP(tensor=ap.tensor, offset=ap.offset, ap=list(ap.ap) + [[0, count]])


def _insert_bcast(ap, count, pos):
    """Insert a zero-stride broadcast dim of length `count` at free-dim
    position `pos` (0 = right after the partition dim)."""
    l = list(ap.ap)
    l.insert(1 + pos, [0, count])
    return bass.AP(tensor=ap.tensor, offset=ap.offset, ap=l)


def _swap_free(ap):
    """Swap the two free dims of a 3D AP (iteration-order change)."""
    l = list(ap.ap)
    assert len(l) == 3
    return bass.AP(tensor=ap.tensor, offset=ap.offset, ap=[l[0], l[2], l[1]])


def build_program(reps=1, **knobs):
    global DMA_J, QSP, QACT, CP_ACT, CP_POOL, SQ_DVE
    for k, v in knobs.items():
        if v is not None:
            globals()[k.upper()] = v
    nc = bacc.Bacc("TRN2", target_bir_lowering=False, debug=False)

    pair = nc.dram_tensor("pair", [IBLK, N, DPAIR], BF16, kind="ExternalInput")
    sT = nc.dram_tensor("sT", [DIM, N], BF16, kind="ExternalInput")
    sTi = nc.dram_tensor("sTi", [DIM, IBLK], BF16, kind="ExternalInput")
    wq = nc.dram_tensor("wq", [DIM, DIM], BF16, kind="ExternalInput")
    wk = nc.dram_tensor("wk", [DIM, DIM], BF16, kind="ExternalInput")
    wv = nc.dram_tensor("wv", [DIM, DIM], BF16, kind="ExternalInput")
    wg = nc.dram_tensor("wg", [DIM, DIM], BF16, kind="ExternalInput")
    wo = nc.dram_tensor("wo", [DIM, DIM], BF16, kind="ExternalInput")
    weff = nc.dram_tensor("weff", [DPAIR, HEADS + 1], BF16, kind="ExternalInput")
    colw = nc.dram_tensor("colw", [128, HEADS], F32, kind="ExternalInput")
    bgt = nc.dram_tensor("bgt", [128, 8], F32, kind="ExternalInput")
    out = nc.dram_tensor("out", [IBLK, DIM], F32, kind="ExternalOutput")

    CT = DIM // 128  # 8 contraction tiles

    with tile.TileContext(nc) as tc:
        with tc.tile_pool(name="consts", bufs=1) as consts, \
             tc.tile_pool(name="persist", bufs=1) as pers:
            ident = consts.tile([128, 128], BF16, tag="ident", name="ident")
            make_identity(nc, ident)
            ones1 = consts.tile([128, 1], BF16, tag="ones1", name="ones1")
            nc.vector.memset(ones1, 1.0)
            onesrow = consts.tile([1, 128], F32, tag="onesrow", name="onesrow")
            nc.vector.memset(onesrow, 1.0)
            weff_sb = consts.tile([DPAIR, HEADS + 1], BF16, tag="weff", name="weff")
            nc.sync.dma_start(out=weff_sb, in_=weff[:, :])
            colw_sb = consts.tile([128, HEADS], F32, tag="colw", name="colw")
            nc.sync.dma_start(out=colw_sb, in_=colw[:, :])
            bgt_sb = consts.tile([128, 8], F32, tag="bgt", name="bgt")
            nc.sync.dma_start(out=bgt_sb, in_=bgt[:, :])
            eps4 = consts.tile([128, 1], F32, tag="eps4", name="eps4")
            nc.vector.memset(eps4, EPS)
            zero1 = consts.tile([128, 1], F32, tag="zero1", name="zero1")
            nc.vector.memset(zero1, 0.0)

            for _rep in range(reps):
                # persistent tensors
                kT = [pers.tile([128, N], BF16, tag=f"kT{t}", name=f"kT{t}") for t in range(8)]
                vsb = [pers.tile([128, DIM], BF16, tag=f"v{t}", name=f"v{t}") for t in range(8)]
                qT = [pers.tile([128, IBLK], BF16, tag=f"qT{t}", name=f"qT{t}") for t in range(8)]
                gT = [pers.tile([128, IBLK], F32, tag=f"gT{t}", name=f"gT{t}") for t in range(8)]
                bias_h = pers.tile([128, HEADS, N], BF16, tag="biasH", name="biasH")
                # wo loads ride the otherwise-idle ACT HWDGE queue during B;
                # persistent tiles so they aren't gated on pb's SBUF freeing.
                wo_sb = [pers.tile([128, DIM], BF16, tag=f"wo{t}", name=f"wo{t}") for t in range(8)]
                for t in range(8):
                    nc.scalar.dma_start(out=wo_sb[t], in_=wo[t * 128:(t + 1) * 128, :])

                # ---------------- Phase B: pairwise LN + bias projection -----
                # (phase A's projections are emitted interleaved, see a_steps)
                with tc.tile_pool(name="pb", bufs=1) as pb, \
                     tc.tile_pool(name="psB", bufs=2, space="PSUM") as psB, \
                     tc.tile_pool(name="pa", bufs=1) as pa, \
                     tc.tile_pool(name="psA", bufs=2, space="PSUM") as psA:
                    # sums/sumsq interleaved: stats[:, j, 0]=sum, [:, j, 1]=sumsq
                    stats = pb.tile([128, N, 2], F32, tag="stats", name="stats")
                    rA = pers.tile([128, N], BF16, tag="rA", name="rA")
                    rm = pers.tile([128, N], BF16, tag="rm", name="rm")

                    def emit_quarter(qi):
                        """LN stats post-pass for j-quarter qi, as soon as its
                        sums/sumsq are complete."""
                        sl = slice(qi * 256, (qi + 1) * 256)
                        mu = pb.tile([128, 256], F32, tag="mu", name="mu", bufs=1)
                        v4 = pb.tile([128, 256], F32, tag="v4", name="v4", bufs=1)
                        d = pb.tile([128, 256], F32, tag="d", name="d", bufs=1)
                        nc.vector.tensor_scalar_mul(out=mu, in0=stats[:, sl, 0],
                                                    scalar1=1.0 / DPAIR)
                        nc.vector.tensor_scalar_mul(out=v4, in0=stats[:, sl, 1],
                                                    scalar1=1.0 / DPAIR)
                        nc.vector.tensor_mul(out=d, in0=mu, in1=mu)
                        nc.vector.tensor_sub(out=v4, in0=v4, in1=d)  # var
                        nc.scalar.activation(out=v4, in_=v4, func=AF.Sqrt,
                                             bias=eps4[:, 0:1], scale=1.0)
                        with nc.allow_low_precision(
                                reason="rinv in bf16: 0.4% rel on LN scale, "
                                       "well inside the 2e-2 gate"):
                            nc.vector.reciprocal(out=rA[:, sl], in_=v4)
                        nc.vector.tensor_mul(out=rm[:, sl], in0=mu, in1=rA[:, sl])
                        # the bias_h affine correction itself is deferred into
                        # phase C's head loop (DVE has slack there)

                    PAIR_J = 16           # j columns per processing unit
                    PAIR_PER_Q = 256 // PAIR_J
                    LAG = 1  # pairs of slack between transpose and pproj/psq

                    pending = []

                    def flush_pair():
                        """Emit pproj/psq + extracts for the oldest pending
                        16-j unit. Lagging these PE ops keeps the in-order PE
                        queue from stalling on the copy / square."""
                        pjc, oct, octsq = pending.pop(0)
                        j0 = pjc * PAIR_J
                        # pproj[:, j, 0:17] = [x@Weff | x@ones]; [:, j, 17] = x^2@ones
                        pproj = psB.tile([128, PAIR_J, HEADS + 2], F32, tag="pproj", bufs=2, name="pproj")
                        for jj in range(PAIR_J):
                            nc.tensor.matmul(pproj[:, jj, 0:HEADS + 1],
                                             oct[:, jj, :], weff_sb,
                                             start=True, stop=True)
                            nc.tensor.matmul(pproj[:, jj, HEADS + 1:HEADS + 2],
                                             octsq[:, jj, :], ones1,
                                             start=True, stop=True)
                        bsl = bias_h[:, :, j0:j0 + PAIR_J]
                        if pjc % 2:
                            nc.scalar.copy(out=_swap_free(bsl),
                                           in_=pproj[:, :, 0:HEADS])
                        else:
                            nc.vector.tensor_copy(out=_swap_free(bsl),
                                                  in_=pproj[:, :, 0:HEADS])
                        nc.vector.tensor_copy(out=stats[:, j0:j0 + PAIR_J, :],
                                              in_=pproj[:, :, HEADS:HEADS + 2])
                        if (pjc + 1) % PAIR_PER_Q == 0:
                            emit_quarter(pjc // PAIR_PER_Q)

                    NCH = N // DMA_J
                    # pairwise chunk -> DMA queue. SP is a free sequencer; its
                    # queue also carries the phase-A weight loads (emitted
                    # early via a_steps). Pool/ACT DMAs block their engine for
                    # the whole transfer, so balance against compute load.
                    x_tiles = {}

                    def issue_dma(dc):
                        x = pb.tile([128, DMA_J, DPAIR], BF16, tag="x", bufs=5, name="x")
                        eng = nc.sync if dc in QSP else (
                            nc.scalar if dc in QACT else nc.gpsimd)
                        eng.dma_start(
                            out=x, in_=pair[:, dc * DMA_J:(dc + 1) * DMA_J, :])
                        x_tiles[dc] = x

                    # ---- phase A, emitted in steps interleaved with B ----
                    def a_steps():
                        s_sb, si_sb = [], []
                        for ct in range(CT):
                            s = pa.tile([128, N], BF16, tag=f"s{ct}", name=f"s{ct}")
                            nc.sync.dma_start(out=s, in_=sT[ct * 128:(ct + 1) * 128, :])
                            s_sb.append(s)
                            si = pa.tile([128, IBLK], BF16, tag=f"si{ct}", name=f"si{ct}")
                            nc.sync.dma_start(out=si, in_=sTi[ct * 128:(ct + 1) * 128, :])
                            si_sb.append(si)
                        yield
                        # kT[t] = (Wk^T @ single^T)[rows t*128...]
                        wsb = [pa.tile([128, DIM], BF16, tag=f"w{ct}", name=f"w{ct}") for ct in range(CT)]
                        for ct in range(CT):
                            nc.sync.dma_start(out=wsb[ct], in_=wk[ct * 128:(ct + 1) * 128, :])
                        yield
                        for t in range(8):
                            for jh in range(2):
                                ps = psA.tile([128, 512], F32, tag="mmA", name="mmA")
                                for ct in range(CT):
                                    nc.tensor.matmul(
                                        ps, wsb[ct][:, t * 128:(t + 1) * 128],
                                        s_sb[ct][:, jh * 512:(jh + 1) * 512],
                                        start=(ct == 0), stop=(ct == CT - 1))
                                dst = kT[t][:, jh * 512:(jh + 1) * 512]
                                nc.scalar.copy(out=dst, in_=ps)
                            yield
                        # v[t] = (single @ Wv)[rows t*128...]   (natural layout)
                        wsb = [pa.tile([128, DIM], BF16, tag=f"w{ct}", name=f"w{ct}") for ct in range(CT)]
                        for ct in range(CT):
                            nc.sync.dma_start(out=wsb[ct], in_=wv[ct * 128:(ct + 1) * 128, :])
                        yield
                        for t in range(8):
                            for vh in range(2):
                                ps = psA.tile([128, 512], F32, tag="mmA", name="mmA")
                                for ct in range(CT):
                                    nc.tensor.matmul(
                                        ps, s_sb[ct][:, t * 128:(t + 1) * 128],
                                        wsb[ct][:, vh * 512:(vh + 1) * 512],
                                        start=(ct == 0), stop=(ct == CT - 1))
                                dst = vsb[t][:, vh * 512:(vh + 1) * 512]
                                nc.scalar.copy(out=dst, in_=ps)
                            yield
                        # qT[t] = (Wq^T @ single^T)[rows t*128, iblk] (Wq pre-scaled)
                        wsb = [pa.tile([128, DIM], BF16, tag=f"w{ct}", name=f"w{ct}") for ct in range(CT)]
                        for ct in range(CT):
                            nc.sync.dma_start(out=wsb[ct], in_=wq[ct * 128:(ct + 1) * 128, :])
                        yield
                        for t in range(8):
                            ps = psA.tile([128, IBLK], F32, tag="mmA", name="mmA")
                            for ct in range(CT):
                                nc.tensor.matmul(
                                    ps, wsb[ct][:, t * 128:(t + 1) * 128], si_sb[ct],
                                    start=(ct == 0), stop=(ct == CT - 1))
                            nc.scalar.copy(out=qT[t], in_=ps)
                            if t % 2:
                                yield
                        # gT[t] = sigmoid((Wg^T @ single^T)[rows t*128, iblk] + bg)
                        wsb = [pa.tile([128, DIM], BF16, tag=f"w{ct}", name=f"w{ct}") for ct in range(CT)]
                        for ct in range(CT):
                            nc.sync.dma_start(out=wsb[ct], in_=wg[ct * 128:(ct + 1) * 128, :])
                        yield
                        for t in range(8):
                            ps = psA.tile([128, IBLK], F32, tag="mmA", name="mmA")
                            for ct in range(CT):
                                nc.tensor.matmul(
                                    ps, wsb[ct][:, t * 128:(t + 1) * 128], si_sb[ct],
                                    start=(ct == 0), stop=(ct == CT - 1))
                            nc.scalar.activation(out=gT[t], in_=ps, func=AF.Sigmoid,
                                                 bias=bgt_sb[:, t:t + 1], scale=1.0)
                            if t % 2:
                                yield

                    agen = a_steps()
                    issue_dma(0)
                    issue_dma(1)
                    issue_dma(2)
                    issue_dma(3)
                    for dc in range(NCH):
                        x = x_tiles.pop(dc)
                        for sc in range(DMA_J // PAIR_J):
                            pjc = dc * (DMA_J // PAIR_J) + sc
                            xs = x[:, sc * PAIR_J:(sc + 1) * PAIR_J, :]
                            poct = psB.tile([128, PAIR_J, 128], BF16, tag="poct", bufs=2, name="poct")
                            for jj in range(PAIR_J):
                                nc.tensor.transpose(poct[:, jj, :], xs[:, jj, :], ident)
                            if sc == 0 and dc + 4 < NCH:
                                issue_dma(dc + 4)
                            oct = pb.tile([128, PAIR_J, 128], BF16, tag="oct", bufs=LAG + 2, name="oct")
                            m = (pjc * 5) % 16  # stride-5 spreads the split
                            if m < CP_ACT:
                                nc.scalar.copy(out=oct.bitcast(F32),
                                               in_=poct.bitcast(F32))
                            else:
                                nc.vector.tensor_copy(out=oct.bitcast(F32),
                                                      in_=poct.bitcast(F32))
                            octsq = pb.tile([128, PAIR_J, 128], BF16, tag="octsq",
                                            bufs=LAG + 1, name="octsq")
                            # all squares read the SBUF copy: dual-PSUM reads
                            # are illegal on DVE, and reading poct on ACT
                            # extends the PSUM tile's lifetime (transpose stalls)
                            if m < SQ_DVE:
                                nc.vector.tensor_mul(out=octsq, in0=oct, in1=oct)
                            else:
                                nc.scalar.activation(out=octsq, in_=oct,
                                                     func=AF.Square)
                            pending.append((pjc, oct, octsq))
                            if len(pending) > LAG:
                                flush_pair()
                            if pjc % 2 == 1:
                                next(agen, None)
                    while pending:
                        flush_pair()
                    for _ in agen:
                        pass

                # ---------------- Phase C: attention (transposed scores) -----
                with tc.tile_pool(name="pc", bufs=1) as pc, \
                     tc.tile_pool(name="psC", bufs=2, space="PSUM") as psC:
                    og = [pc.tile([128, IBLK], BF16, tag=f"og{t}", name=f"og{t}") for t in range(8)]

                    # Software-pipelined over heads: head h's ssum/av (PE ops
                    # that wait on exp_h) are emitted AFTER head h+1's kq/bias
                    # matmuls, so the in-order PE queue never stalls on ACT.
                    state = {}  # t -> (rsb, ot_ps)

                    def finish_head(h, expT):
                        t = h // 2
                        off = 64 * (h % 2)
                        if h % 2 == 0:
                            rsb = psC.tile([128, IBLK], F32, tag="ot", bufs=2, name="rsb")
                            ot_ps = psC.tile([128, IBLK], F32, tag="ot", bufs=2, name="ot")
                            state[t] = (rsb, ot_ps)
                        rsb, ot_ps = state[t]
                        ssb = psC.tile([1, 128], F32, tag="ssb", bufs=2, name="ssb")
                        for jb in range(8):
                            nc.tensor.matmul(ssb, ones1, expT[:, jb, :],
                                             start=(jb == 0), stop=(jb == 7))
                        rs = pc.tile([1, 128], F32, tag="rs", bufs=3, name="rs")
                        nc.vector.reciprocal(out=rs, in_=ssb)
                        nc.tensor.matmul(rsb[off:off + 64, :],
                                         onesrow[:, 0:64], rs,
                                         start=True, stop=True)
                        for jt in range(8):
                            nc.tensor.matmul(
                                ot_ps[off:off + 64, :],
                                vsb[jt][:, h * 64:(h + 1) * 64], expT[:, jt, :],
                                start=(jt == 0), stop=(jt == 7))
                        if h % 2 == 1:
                            nc.vector.tensor_mul(out=gT[t], in0=gT[t], in1=rsb)
                            nc.vector.tensor_mul(out=og[t], in0=ot_ps, in1=gT[t])

                    prev = None
                    SL3 = slice(0, 1024)
                    for h in range(HEADS):
                        t = h // 2
                        off = 64 * (h % 2)
                        # deferred LN correction of quarter 3 for this head
                        nc.vector.tensor_mul(out=bias_h[:, h, SL3],
                                             in0=bias_h[:, h, SL3],
                                             in1=rA[:, SL3])
                        t2h = pc.tile([128, 1024], BF16, tag="t2h", name="t2h",
                                      bufs=2)
                        nc.vector.tensor_scalar_mul(
                            out=t2h, in0=rm[:, SL3],
                            scalar1=colw_sb[:, h:h + 1])
                        nc.vector.tensor_add(out=bias_h[:, h, SL3],
                                             in0=bias_h[:, h, SL3], in1=t2h)
                        # scT[j, i] for j-block jb: k^T q + bias^T (identity-rhs)
                        scT = psC.tile([128, 8, 128], F32, tag="scT", bufs=2, name="scT")
                        for jb in range(8):
                            nc.tensor.matmul(
                                scT[:, jb, :],
                                kT[t][off:off + 64, jb * 128:(jb + 1) * 128],
                                qT[t][off:off + 64, :], start=True, stop=False)
                            nc.tensor.matmul(
                                scT[:, jb, :],
                                bias_h[:, h, jb * 128:(jb + 1) * 128], ident,
                                start=False, stop=True)
                        # scores are O(10): exp without max-subtraction is safe in
                        # f32/bf16 range, softmax is shift-invariant.
                        expT = pc.tile([128, 8, 128], BF16, tag="expT", bufs=3, name="expT")
                        nc.scalar.activation(out=expT, in_=scT, func=AF.Exp,
                                             bias=zero1[:, 0:1], scale=1.0)
                        if prev is not None:
                            finish_head(*prev)
                        prev = (h, expT)
                    finish_head(*prev)

                    # out = og^T @ Wo
                    out_sb = pc.tile([128, DIM], F32, tag="out_sb", name="out_sb")
                    for eh in range(2):
                        ps = psC.tile([128, 512], F32, tag="scT", bufs=2, name="po")
                        for t in range(8):
                            nc.tensor.matmul(
                                ps, og[t], wo_sb[t][:, eh * 512:(eh + 1) * 512],
                                start=(t == 0), stop=(t == 7))
                        nc.scalar.copy(out=out_sb[:, eh * 512:(eh + 1) * 512], in_=ps)
                    nc.sync.dma_start(out=out[:, :], in_=out_sb)

    nc.compile()
    return nc


_CACHE = {}


def _prep_inputs(single_repr, pairwise_repr, ln_gamma, ln_beta, W_bias,
                 Wq, Wk, Wv, Wg, bg, Wo):
    sr = np.asarray(single_repr, np.float32).reshape(N, DIM)
    pw = np.asarray(pairwise_repr, np.float32).reshape(N, N, DPAIR).astype(BFNP)
    gamma = np.asarray(ln_gamma, np.float32)
    Wb = np.asarray(W_bias, np.float32)
    weff = gamma[:, None] * Wb                                   # [128, 16]
    colw = np.ascontiguousarray(
        np.broadcast_to(-weff.sum(0)[None, :], (128, HEADS))).astype(np.float32)
    weff17 = np.concatenate(
        [weff, np.ones((DPAIR, 1), np.float32)], axis=1)         # [128, 17]
    sT_np = np.ascontiguousarray(sr.T).astype(BFNP)              # [DIM, N]
    scale = DHEAD ** -0.5
    common = {
        "sT": sT_np,
        "wq": (np.asarray(Wq, np.float32) * scale).astype(BFNP),
        "wk": np.asarray(Wk, np.float32).astype(BFNP),
        "wv": np.asarray(Wv, np.float32).astype(BFNP),
        "wg": np.asarray(Wg, np.float32).astype(BFNP),
        "wo": np.asarray(Wo, np.float32).astype(BFNP),
        "weff": weff17.astype(BFNP),
        "colw": colw,
        "bgt": np.ascontiguousarray(
            np.asarray(bg, np.float32).reshape(8, 128).T),
    }
    in_maps = []
    for c in range(NCORES):
        m = dict(common)
        m["pair"] = pw[c * IBLK:(c + 1) * IBLK]
        m["sTi"] = np.ascontiguousarray(sT_np[:, c * IBLK:(c + 1) * IBLK])
        in_maps.append(m)
    return in_maps


def kernel(single_repr, pairwise_repr, ln_gamma, ln_beta, W_bias,
           Wq, Wk, Wv, Wg, bg, Wo, _trace=False):
    if "nc" not in _CACHE:
        _CACHE["nc"] = build_program()
    nc = _CACHE["nc"]
    in_maps = _prep_inputs(single_repr, pairwise_repr, ln_gamma, ln_beta,
                           W_bias, Wq, Wk, Wv, Wg, bg, Wo)
    res = run_bass_kernel_spmd(nc, in_maps, core_ids=list(range(NCORES)),
                               trace=_trace)
    out = np.concatenate([res.results[c]["out"] for c in range(NCORES)], axis=0)
    if _trace:
        kernel.last_result = res
    return out.reshape(1, N, DIM).astype(np.float32)


# revision 49
# speedup vs baseline: 1.1753x; 1.0072x over previous
"""AttentionPairBias kernel for Trainium2, 8-core SPMD.

Math (per batch=1):
  pn        = LayerNorm(pairwise) * gamma + beta                  [N, N, 128]
  attn_bias = einsum('ijp,ph->hij', pn, W_bias)                   [16, N, N]
  q,k,v     = single @ Wq/Wk/Wv  (split into 16 heads of 64)
  scores    = q k^T / sqrt(64) + attn_bias ; attn = softmax_j
  o         = attn @ v ; out = (o * sigmoid(single@Wg + bg)) @ Wo [N, 1024]

Sharding: rows of i (queries) across 8 cores; k/v compute replicated.

Engine plan: pairwise is pre-cast to bf16 on the host and streamed over
BOTH the SP (HWDGE) and Pool (SWDGE) DMA queues; PE transposes it to
[p,(j,i)]. PSUM->SBUF copies and squares split ACT/DVE (GPSIMD cannot
touch PSUM and its compute is slow on real HW, so Pool stays DMA-only);
LN stats reduce on PE via ones-matmul columns of the projection. The
q/k/v/g projections are emitted interleaved with the pairwise stream
(a_steps) so their matmuls fill PE while DMA paces phase B. Attention
runs with TRANSPOSED scores [j,i]: the softmax sum and the pair-bias add
both become PE matmuls (bias^T via identity-rhs matmul), so attn@v needs
no transpose and ACT only does one exp per head; 1/sum is folded into
the gate via a PE row-broadcast. The last j-quarter's LN correction is
deferred into the head loop to keep it off the B->C critical path.
LayerNorm is folded as a post-projection affine:
  bias[i,j,h] = rinv*(x@W_eff) - (rinv*mu)*colsum(W_eff)
(+beta@W_bias is constant over j so it cancels in softmax).
"""

import numpy as np
import ml_dtypes

import concourse.bacc as bacc
import concourse.bass as bass
import concourse.tile as tile
import concourse.mybir as mybir
from concourse.bass_utils import run_bass_kernel_spmd
from concourse.masks import make_identity

N, DIM, HEADS, DHEAD, DPAIR = 1024, 1024, 16, 64, 128
NCORES = 8
IBLK = N // NCORES  # 128
EPS = 1e-5

# tuning knobs
DMA_J = 32        # j-columns per pairwise DMA
QSP = frozenset(range(7, 32, 2))  # late odd chunks on SP (weights go first), rest Pool
QACT = frozenset()                # pairwise chunks on ACT queue
CP_ACT = 9        # pair copies: pjc%16 < CP_ACT -> ACT else DVE
SQ_DVE = 12       # pair squares: pjc%16 < SQ_DVE -> DVE else ACT

F32 = mybir.dt.float32
BF16 = mybir.dt.bfloat16
AX = mybir.AxisListType
AF = mybir.ActivationFunctionType
BFNP = ml_dtypes.bfloat16


def _bcast_free(ap, count, where=-1):
    """Append a zero-stride broadcast dim of length `count` to an AP."""
    return bass.AP(tensor=ap.tensor, offset=ap.offset, ap=list(ap.ap) + [[0, count]])


def _insert_bcast(ap, count, pos):
    """Insert a zero-stride broadcast dim of length `count` at free-dim
    position `pos` (0 = right after the partition dim)."""
    l = list(ap.ap)
    l.insert(1 + pos, [0, count])
    return bass.AP(tensor=ap.tensor, offset=ap.offset, ap=l)


def _swap_free(ap):
    """Swap the two free dims of a 3D AP (iteration-order change)."""
    l = list(ap.ap)
    assert len(l) == 3
    return bass.AP(tensor=ap.tensor, offset=ap.offset, ap=[l[0], l[2], l[1]])


def build_program(reps=1, **knobs):
    global DMA_J, QSP, QACT, CP_ACT, CP_POOL, SQ_DVE
    for k, v in knobs.items():
        if v is not None:
            globals()[k.upper()] = v
    nc = bacc.Bacc("TRN2", target_bir_lowering=False, debug=False)

    pair = nc.dram_tensor("pair", [IBLK, N, DPAIR], BF16, kind="ExternalInput")
    sT = nc.dram_tensor("sT", [DIM, N], BF16, kind="ExternalInput")
    sTi = nc.dram_tensor("sTi", [DIM, IBLK], BF16, kind="ExternalInput")
    wq = nc.dram_tensor("wq", [DIM, DIM], BF16, kind="ExternalInput")
    wk = nc.dram_tensor("wk", [DIM, DIM], BF16, kind="ExternalInput")
    wv = nc.dram_tensor("wv", [DIM, DIM], BF16, kind="ExternalInput")
    wg = nc.dram_tensor("wg", [DIM, DIM], BF16, kind="ExternalInput")
    wo = nc.dram_tensor("wo", [DIM, DIM], BF16, kind="ExternalInput")
    weff = nc.dram_tensor("weff", [DPAIR, HEADS + 1], BF16, kind="ExternalInput")
    colw = nc.dram_tensor("colw", [128, HEADS], F32, kind="ExternalInput")
    bgt = nc.dram_tensor("bgt", [128, 8], F32, kind="ExternalInput")
    out = nc.dram_tensor("out", [IBLK, DIM], F32, kind="ExternalOutput")

    CT = DIM // 128  # 8 contraction tiles

    with tile.TileContext(nc) as tc:
        with tc.tile_pool(name="consts", bufs=1) as consts, \
             tc.tile_pool(name="persist", bufs=1) as pers:
            ident = consts.tile([128, 128], BF16, tag="ident", name="ident")
            make_identity(nc, ident)
            ones1 = consts.tile([128, 1], BF16, tag="ones1", name="ones1")
            nc.vector.memset(ones1, 1.0)
            onesrow = consts.tile([1, 128], F32, tag="onesrow", name="onesrow")
            nc.vector.memset(onesrow, 1.0)
            weff_sb = consts.tile([DPAIR, HEADS + 1], BF16, tag="weff", name="weff")
            nc.sync.dma_start(out=weff_sb, in_=weff[:, :])
            colw_sb = consts.tile([128, HEADS], F32, tag="colw", name="colw")
            nc.sync.dma_start(out=colw_sb, in_=colw[:, :])
            bgt_sb = consts.tile([128, 8], F32, tag="bgt", name="bgt")
            nc.sync.dma_start(out=bgt_sb, in_=bgt[:, :])
            eps4 = consts.tile([128, 1], F32, tag="eps4", name="eps4")
            nc.vector.memset(eps4, EPS)
            zero1 = consts.tile([128, 1], F32, tag="zero1", name="zero1")
            nc.vector.memset(zero1, 0.0)

            for _rep in range(reps):
                # persistent tensors
                kT = [pers.tile([128, N], BF16, tag=f"kT{t}", name=f"kT{t}") for t in range(8)]
                vsb = [pers.tile([128, DIM], BF16, tag=f"v{t}", name=f"v{t}") for t in range(8)]
                qT = [pers.tile([128, IBLK], BF16, tag=f"qT{t}", name=f"qT{t}") for t in range(8)]
                gT = [pers.tile([128, IBLK], F32, tag=f"gT{t}", name=f"gT{t}") for t in range(8)]
                bias_h = pers.tile([128, HEADS, N], BF16, tag="biasH", name="biasH")
                # wo loads ride the otherwise-idle ACT HWDGE queue during B;
                # persistent tiles so they aren't gated on pb's SBUF freeing.
                wo_sb = [pers.tile([128, DIM], BF16, tag=f"wo{t}", name=f"wo{t}") for t in range(8)]
                for t in range(8):
                    nc.scalar.dma_start(out=wo_sb[t], in_=wo[t * 128:(t + 1) * 128, :])

                # ---------------- Phase B: pairwise LN + bias projection -----
                # (phase A's projections are emitted interleaved, see a_steps)
                with tc.tile_pool(name="pb", bufs=1) as pb, \
                     tc.tile_pool(name="psB", bufs=2, space="PSUM") as psB, \
                     tc.tile_pool(name="pa", bufs=1) as pa, \
                     tc.tile_pool(name="psA", bufs=2, space="PSUM") as psA:
                    # sums/sumsq interleaved: stats[:, j, 0]=sum, [:, j, 1]=sumsq
                    stats = pb.tile([128, N, 2], F32, tag="stats", name="stats")
                    rA = pers.tile([128, N], BF16, tag="rA", name="rA")
                    rm = pers.tile([128, N], BF16, tag="rm", name="rm")

                    def emit_quarter(qi):
                        """LN stats post-pass for j-quarter qi, as soon as its
                        sums/sumsq are complete."""
                        sl = slice(qi * 256, (qi + 1) * 256)
                        mu = pb.tile([128, 256], F32, tag="mu", name="mu", bufs=1)
                        v4 = pb.tile([128, 256], F32, tag="v4", name="v4", bufs=1)
                        d = pb.tile([128, 256], F32, tag="d", name="d", bufs=1)
                        nc.vector.tensor_scalar_mul(out=mu, in0=stats[:, sl, 0],
                                                    scalar1=1.0 / DPAIR)
                        nc.vector.tensor_scalar_mul(out=v4, in0=stats[:, sl, 1],
                                                    scalar1=1.0 / DPAIR)
                        nc.vector.tensor_mul(out=d, in0=mu, in1=mu)
                        nc.vector.tensor_sub(out=v4, in0=v4, in1=d)  # var
                        nc.scalar.activation(out=v4, in_=v4, func=AF.Sqrt,
                                             bias=eps4[:, 0:1], scale=1.0)
                        with nc.allow_low_precision(
                                reason="rinv in bf16: 0.4% rel on LN scale, "
                                       "well inside the 2e-2 gate"):
                            nc.vector.reciprocal(out=rA[:, sl], in_=v4)
                        nc.vector.tensor_mul(out=rm[:, sl], in0=mu, in1=rA[:, sl])
                        # the bias_h affine correction itself is deferred into
                        # phase C's head loop (DVE has slack there)

                    PAIR_J = 16           # j columns per processing unit
                    PAIR_PER_Q = 256 // PAIR_J
                    LAG = 1  # pairs of slack between transpose and pproj/psq

                    pending = []

                    def flush_pair():
                        """Emit pproj/psq + extracts for the oldest pending
                        16-j unit. Lagging these PE ops keeps the in-order PE
                        queue from stalling on the copy / square."""
                        pjc, oct, octsq = pending.pop(0)
                        j0 = pjc * PAIR_J
                        # pproj[:, j, 0:17] = [x@Weff | x@ones]; [:, j, 17] = x^2@ones
                        pproj = psB.tile([128, PAIR_J, HEADS + 2], F32, tag="pproj", bufs=2, name="pproj")
                        for jj in range(PAIR_J):
                            nc.tensor.matmul(pproj[:, jj, 0:HEADS + 1],
                                             oct[:, jj, :], weff_sb,
                                             start=True, stop=True)
                            nc.tensor.matmul(pproj[:, jj, HEADS + 1:HEADS + 2],
                                             octsq[:, jj, :], ones1,
                                             start=True, stop=True)
                        bsl = bias_h[:, :, j0:j0 + PAIR_J]
                        if pjc % 2:
                            nc.scalar.copy(out=_swap_free(bsl),
                                           in_=pproj[:, :, 0:HEADS])
                        else:
                            nc.vector.tensor_copy(out=_swap_free(bsl),
                                                  in_=pproj[:, :, 0:HEADS])
                        nc.vector.tensor_copy(out=stats[:, j0:j0 + PAIR_J, :],
                                              in_=pproj[:, :, HEADS:HEADS + 2])
                        if (pjc + 1) % PAIR_PER_Q == 0:
                            emit_quarter(pjc // PAIR_PER_Q)

                    NCH = N // DMA_J
                    # pairwise chunk -> DMA queue. SP is a free sequencer; its
                    # queue also carries the phase-A weight loads (emitted
                    # early via a_steps). Pool/ACT DMAs block their engine for
                    # the whole transfer, so balance against compute load.
                    x_tiles = {}

                    def issue_dma(dc):
                        x = pb.tile([128, DMA_J, DPAIR], BF16, tag="x", bufs=5, name="x")
                        eng = nc.sync if dc in QSP else (
                            nc.scalar if dc in QACT else nc.gpsimd)
                        eng.dma_start(
                            out=x, in_=pair[:, dc * DMA_J:(dc + 1) * DMA_J, :])
                        x_tiles[dc] = x

                    # ---- phase A, emitted in steps interleaved with B ----
                    def a_steps():
                        s_sb, si_sb = [], []
                        for ct in range(CT):
                            s = pa.tile([128, N], BF16, tag=f"s{ct}", name=f"s{ct}")
                            nc.sync.dma_start(out=s, in_=sT[ct * 128:(ct + 1) * 128, :])
                            s_sb.append(s)
                            si = pa.tile([128, IBLK], BF16, tag=f"si{ct}", name=f"si{ct}")
                            nc.sync.dma_start(out=si, in_=sTi[ct * 128:(ct + 1) * 128, :])
                            si_sb.append(si)
                        yield
                        # kT[t] = (Wk^T @ single^T)[rows t*128...]
                        wsb = [pa.tile([128, DIM], BF16, tag=f"w{ct}", name=f"w{ct}") for ct in range(CT)]
                        for ct in range(CT):
                            nc.sync.dma_start(out=wsb[ct], in_=wk[ct * 128:(ct + 1) * 128, :])
                        yield
                        for t in range(8):
                            for jh in range(2):
                                ps = psA.tile([128, 512], F32, tag="mmA", name="mmA")
                                for ct in range(CT):
                                    nc.tensor.matmul(
                                        ps, wsb[ct][:, t * 128:(t + 1) * 128],
                                        s_sb[ct][:, jh * 512:(jh + 1) * 512],
                                        start=(ct == 0), stop=(ct == CT - 1))
                                dst = kT[t][:, jh * 512:(jh + 1) * 512]
                                nc.scalar.copy(out=dst, in_=ps)
                            yield
                        # v[t] = (single @ Wv)[rows t*128...]   (natural layout)
                        wsb = [pa.tile([128, DIM], BF16, tag=f"w{ct}", name=f"w{ct}") for ct in range(CT)]
                        for ct in range(CT):
                            nc.sync.dma_start(out=wsb[ct], in_=wv[ct * 128:(ct + 1) * 128, :])
                        yield
                        for t in range(8):
                            for vh in range(2):
                                ps = psA.tile([128, 512], F32, tag="mmA", name="mmA")
                                for ct in range(CT):
                                    nc.tensor.matmul(
                                        ps, s_sb[ct][:, t * 128:(t + 1) * 128],
                                        wsb[ct][:, vh * 512:(vh + 1) * 512],
                                        start=(ct == 0), stop=(ct == CT - 1))
                                dst = vsb[t][:, vh * 512:(vh + 1) * 512]
                                nc.scalar.copy(out=dst, in_=ps)
                            yield
                        # qT[t] = (Wq^T @ single^T)[rows t*128, iblk] (Wq pre-scaled)
                        wsb = [pa.tile([128, DIM], BF16, tag=f"w{ct}", name=f"w{ct}") for ct in range(CT)]
                        for ct in range(CT):
                            nc.sync.dma_start(out=wsb[ct], in_=wq[ct * 128:(ct + 1) * 128, :])
                        yield
                        for t in range(8):
                            ps = psA.tile([128, IBLK], F32, tag="mmA", name="mmA")
                            for ct in range(CT):
                                nc.tensor.matmul(
                                    ps, wsb[ct][:, t * 128:(t + 1) * 128], si_sb[ct],
                                    start=(ct == 0), stop=(ct == CT - 1))
                            nc.scalar.copy(out=qT[t], in_=ps)
                            if t % 2:
                                yield
                        # gT[t] = sigmoid((Wg^T @ single^T)[rows t*128, iblk] + bg)
                        wsb = [pa.tile([128, DIM], BF16, tag=f"w{ct}", name=f"w{ct}") for ct in range(CT)]
                        for ct in range(CT):
                            nc.sync.dma_start(out=wsb[ct], in_=wg[ct * 128:(ct + 1) * 128, :])
                        yield
                        for t in range(8):
                            ps = psA.tile([128, IBLK], F32, tag="mmA", name="mmA")
                            for ct in range(CT):
                                nc.tensor.matmul(
                                    ps, wsb[ct][:, t * 128:(t + 1) * 128], si_sb[ct],
                                    start=(ct == 0), stop=(ct == CT - 1))
                            nc.scalar.activation(out=gT[t], in_=ps, func=AF.Sigmoid,
                                                 bias=bgt_sb[:, t:t + 1], scale=1.0)
                            if t % 2:
                                yield

                    agen = a_steps()
                    issue_dma(0)
                    issue_dma(1)
                    issue_dma(2)
                    issue_dma(3)
                    for dc in range(NCH):
                        x = x_tiles.pop(dc)
                        for sc in range(DMA_J // PAIR_J):
                            pjc = dc * (DMA_J // PAIR_J) + sc
                            xs = x[:, sc * PAIR_J:(sc + 1) * PAIR_J, :]
                            poct = psB.tile([128, PAIR_J, 128], BF16, tag="poct", bufs=2, name="poct")
                            for jj in range(PAIR_J):
                                nc.tensor.transpose(poct[:, jj, :], xs[:, jj, :], ident)
                            if sc == 0 and dc + 4 < NCH:
                                issue_dma(dc + 4)
                            oct = pb.tile([128, PAIR_J, 128], BF16, tag="oct", bufs=LAG + 2, name="oct")
                            m = (pjc * 5) % 16  # stride-5 spreads the split
                            if m < CP_ACT:
                                nc.scalar.copy(out=oct.bitcast(F32),
                                               in_=poct.bitcast(F32))
                            else:
                                nc.vector.tensor_copy(out=oct.bitcast(F32),
                                                      in_=poct.bitcast(F32))
                            octsq = pb.tile([128, PAIR_J, 128], BF16, tag="octsq",
                                            bufs=LAG + 1, name="octsq")
                            # all squares read the SBUF copy: dual-PSUM reads
                            # are illegal on DVE, and reading poct on ACT
                            # extends the PSUM tile's lifetime (transpose stalls)
                            if m < SQ_DVE:
                                nc.vector.tensor_mul(out=octsq, in0=oct, in1=oct)
                            else:
                                nc.scalar.activation(out=octsq, in_=oct,
                                                     func=AF.Square)
                            pending.append((pjc, oct, octsq))
                            if len(pending) > LAG:
                                flush_pair()
                            if pjc % 2 == 1:
                                next(agen, None)
                    while pending:
                        flush_pair()
                    for _ in agen:
                        pass

                # ---------------- Phase C: attention (transposed scores) -----
                with tc.tile_pool(name="pc", bufs=1) as pc, \
                     tc.tile_pool(name="psC", bufs=2, space="PSUM") as psC:
                    og = [pc.tile([128, IBLK], BF16, tag=f"og{t}", name=f"og{t}") for t in range(8)]

                    # Software-pipelined over heads: head h's ssum/av (PE ops
                    # that wait on exp_h) are emitted AFTER head h+1's kq/bias
                    # matmuls, so the in-order PE queue never stalls on ACT.
                    state = {}  # t -> (rsb, ot_ps)

                    def finish_head(h, expT):
                        t = h // 2
                        off = 64 * (h % 2)
                        if h % 2 == 0:
                            rsb = psC.tile([128, IBLK], F32, tag="ot", bufs=2, name="rsb")
                            ot_ps = psC.tile([128, IBLK], F32, tag="ot", bufs=2, name="ot")
                            state[t] = (rsb, ot_ps)
                        rsb, ot_ps = state[t]
                        ssb = psC.tile([1, 128], F32, tag="ssb", bufs=2, name="ssb")
                        for jb in range(8):
                            nc.tensor.matmul(ssb, ones1, expT[:, jb, :],
                                             start=(jb == 0), stop=(jb == 7))
                        rs = pc.tile([1, 128], F32, tag="rs", bufs=3, name="rs")
                        nc.vector.reciprocal(out=rs, in_=ssb)
                        nc.tensor.matmul(rsb[off:off + 64, :],
                                         onesrow[:, 0:64], rs,
                                         start=True, stop=True)
                        for jt in range(8):
                            nc.tensor.matmul(
                                ot_ps[off:off + 64, :],
                                vsb[jt][:, h * 64:(h + 1) * 64], expT[:, jt, :],
                                start=(jt == 0), stop=(jt == 7))
                        if h % 2 == 1:
                            nc.vector.tensor_mul(out=gT[t], in0=gT[t], in1=rsb)
                            nc.vector.tensor_mul(out=og[t], in0=ot_ps, in1=gT[t])

                    prev = None
                    SL3 = slice(0, 1024)
                    for h in range(HEADS):
                        t = h // 2
                        off = 64 * (h % 2)
                        # deferred LN correction of quarter 3 for this head
                        nc.vector.tensor_mul(out=bias_h[:, h, SL3],
                                             in0=bias_h[:, h, SL3],
                                             in1=rA[:, SL3])
                        t2h = pc.tile([128, 1024], BF16, tag="t2h", name="t2h",
                                      bufs=2)
                        nc.vector.tensor_scalar_mul(
                            out=t2h, in0=rm[:, SL3],
                            scalar1=colw_sb[:, h:h + 1])
                        nc.vector.tensor_add(out=bias_h[:, h, SL3],
                                             in0=bias_h[:, h, SL3], in1=t2h)
                        # scT[j, i] for j-block jb: k^T q + bias^T (identity-rhs)
                        scT = psC.tile([128, 8, 128], F32, tag="scT", bufs=2, name="scT")
                        for jb in range(8):
                            nc.tensor.matmul(
                                scT[:, jb, :],
                                kT[t][off:off + 64, jb * 128:(jb + 1) * 128],
                                qT[t][off:off + 64, :], start=True, stop=False)
                            nc.tensor.matmul(
                                scT[:, jb, :],
                                bias_h[:, h, jb * 128:(jb + 1) * 128], ident,
                                start=False, stop=True)
                        # scores are O(10): exp without max-subtraction is safe in
                        # f32/bf16 range, softmax is shift-invariant.
                        expT = pc.tile([128, 8, 128], BF16, tag="expT", bufs=3, name="expT")
                        nc.scalar.activation(out=expT, in_=scT, func=AF.Exp,
                                             bias=zero1[:, 0:1], scale=1.0)
                        if prev is not None:
                            finish_head(*prev)
                        prev = (h, expT)
                    finish_head(*prev)

                    # out = og^T @ Wo; store per half so the first DMA
                    # overlaps the second half's copy
                    out_sb = pc.tile([128, DIM], F32, tag="out_sb", name="out_sb")
                    for eh in range(2):
                        ps = psC.tile([128, 512], F32, tag="scT", bufs=2, name="po")
                        for t in range(8):
                            nc.tensor.matmul(
                                ps, og[t], wo_sb[t][:, eh * 512:(eh + 1) * 512],
                                start=(t == 0), stop=(t == 7))
                        nc.scalar.copy(out=out_sb[:, eh * 512:(eh + 1) * 512], in_=ps)
                        nc.sync.dma_start(out=out[:, eh * 512:(eh + 1) * 512],
                                          in_=out_sb[:, eh * 512:(eh + 1) * 512])

    nc.compile()
    return nc


_CACHE = {}


def _prep_inputs(single_repr, pairwise_repr, ln_gamma, ln_beta, W_bias,
                 Wq, Wk, Wv, Wg, bg, Wo):
    sr = np.asarray(single_repr, np.float32).reshape(N, DIM)
    pw = np.asarray(pairwise_repr, np.float32).reshape(N, N, DPAIR).astype(BFNP)
    gamma = np.asarray(ln_gamma, np.float32)
    Wb = np.asarray(W_bias, np.float32)
    weff = gamma[:, None] * Wb                                   # [128, 16]
    colw = np.ascontiguousarray(
        np.broadcast_to(-weff.sum(0)[None, :], (128, HEADS))).astype(np.float32)
    weff17 = np.concatenate(
        [weff, np.ones((DPAIR, 1), np.float32)], axis=1)         # [128, 17]
    sT_np = np.ascontiguousarray(sr.T).astype(BFNP)              # [DIM, N]
    scale = DHEAD ** -0.5
    common = {
        "sT": sT_np,
        "wq": (np.asarray(Wq, np.float32) * scale).astype(BFNP),
        "wk": np.asarray(Wk, np.float32).astype(BFNP),
        "wv": np.asarray(Wv, np.float32).astype(BFNP),
        "wg": np.asarray(Wg, np.float32).astype(BFNP),
        "wo": np.asarray(Wo, np.float32).astype(BFNP),
        "weff": weff17.astype(BFNP),
        "colw": colw,
        "bgt": np.ascontiguousarray(
            np.asarray(bg, np.float32).reshape(8, 128).T),
    }
    in_maps = []
    for c in range(NCORES):
        m = dict(common)
        m["pair"] = pw[c * IBLK:(c + 1) * IBLK]
        m["sTi"] = np.ascontiguousarray(sT_np[:, c * IBLK:(c + 1) * IBLK])
        in_maps.append(m)
    return in_maps


def kernel(single_repr, pairwise_repr, ln_gamma, ln_beta, W_bias,
           Wq, Wk, Wv, Wg, bg, Wo, _trace=False):
    if "nc" not in _CACHE:
        _CACHE["nc"] = build_program()
    nc = _CACHE["nc"]
    in_maps = _prep_inputs(single_repr, pairwise_repr, ln_gamma, ln_beta,
                           W_bias, Wq, Wk, Wv, Wg, bg, Wo)
    res = run_bass_kernel_spmd(nc, in_maps, core_ids=list(range(NCORES)),
                               trace=_trace)
    out = np.concatenate([res.results[c]["out"] for c in range(NCORES)], axis=0)
    if _trace:
        kernel.last_result = res
    return out.reshape(1, N, DIM).astype(np.float32)
